# revision 33
# baseline (speedup 1.0000x reference)
"""Causal self-attention on 8 trn2 NeuronCores.

Sharding: (batch, head-half) per core. Core c handles batch b=c//2 and
heads hh*8..hh*8+7 where hh=c%2. QKV projection + attention run fully
local in bf16; the two cores of a batch exchange attention-output halves
with pair AllGathers (two per 512-token q-tile, staged per pair-half so
the exchange hides behind later compute); out-projection is
column-parallel within the pair (512 output cols/core); host assembles.

Schedule: the attention inner loop is scalar-engine (exp) bound, so QKV
projection chunks of the next t-tile and out-projection chunks of the
previous q-tile are interleaved between attention score/AV groups to
keep the PE stream dense (avoids HAM down-throttle).

Layout per core (pairs p=0..3, local heads 2p, 2p+1):
  Q2T/K2T[p] [128, T] bf16  transposed q/k head-dim-major.
  VN[head]   [128, 16, 65] bf16 V k-chunks + ones column so the AV
             matmul also emits the softmax row-sum at psum partition 64.
  scores     S^T chunk [128 k, <=512 q] f32 psum; diagonal chunks are
             computed sliced ([o:512]) instead of masked+memset.
  E^T        exp(S/8) bf16 via ACT; band-mask mult on diagonal block.
  attnall[p] [128, T] bf16 unnormalized; batched reciprocal + ones
             broadcast matmul normalize in place per pair-half.
W_out is row-permuted host-side so out-projection contraction chunks
0..3 come from the pairs-{0,1} AllGather and 4..7 from pairs-{2,3}.
"""

import numpy as np

D_MODEL = 1024
N_HEADS = 16
HEAD_DIM = 64
B = 4
T = 2048
N_CORES = 8
TQ = 512          # q tile
KC = 128          # k chunk
GROUP = 2         # k-chunks per exp group
NT = T // TQ      # q tiles per batch (4)
NKC = T // KC     # k chunks per batch (16)
NCC = D_MODEL // 128  # contraction chunks (8)
NPAIR = 4         # head pairs per core (8 heads)
MODEL_NO_COLLECTIVE = False  # timing-analysis only: swap AG for local DMA

_PROGRAM_CACHE = {}


def _split_multi_waits(nc, max_waits=1):
    """This toolchain's walrus encodes at most one sync-wait per
    instruction; hoist excess waits onto same-engine carrier nops."""
    import concourse.mybir as mybir

    ctr = 0
    for f in nc.m.functions:
        new_blocks = []
        for bb in f.blocks:
            insts = list(bb.instructions)
            if not any(
                inst.sync_info is not None and len(inst.sync_info.on_wait) > max_waits
                for inst in insts
            ):
                new_blocks.append(bb)
                continue
            out = []
            for inst in insts:
                si = inst.sync_info
                if si is not None and len(si.on_wait) > max_waits:
                    waits = list(si.on_wait)
                    excess = waits[max_waits:]
                    while excess:
                        ctr += 1
                        nop = mybir.InstNoOp(
                            name=f"waitcarrier-{ctr}", engine=inst.engine
                        )
                        nop.sync_info = mybir.SyncInfo(
                            on_wait=excess[:max_waits], on_update=[]
                        )
                        out.append(nop)
                        excess = excess[max_waits:]
                    si.on_wait = waits[:max_waits]
                out.append(inst)
            nb = mybir.BasicBlock(
                name=bb.name,
                instructions=[],
                IsPredicated=bb.IsPredicated,
                IsExit=bb.IsExit,
                IsLoopEntry=bb.IsLoopEntry,
            )
            for i in out:
                nb.add_instruction(i)
            new_blocks.append(nb)
        f.blocks = new_blocks


def _build_program():
    import concourse.bass as bass
    import concourse.mybir as mybir
    import concourse.tile as tile
    from contextlib import ExitStack

    f32 = mybir.dt.float32
    bf16 = mybir.dt.bfloat16
    nc = bass.Bass()

    xT = nc.declare_dram_parameter("xT", [D_MODEL, T], bf16, isOutput=False)
    wq = nc.declare_dram_parameter("wq", [D_MODEL, TQ], bf16, isOutput=False)
    wk = nc.declare_dram_parameter("wk", [D_MODEL, TQ], bf16, isOutput=False)
    wv = nc.declare_dram_parameter("wv", [D_MODEL, TQ], bf16, isOutput=False)
    wo = nc.declare_dram_parameter("wo", [D_MODEL, TQ], bf16, isOutput=False)
    band_in = nc.declare_dram_parameter("band", [128, 128], bf16, isOutput=False)
    ident_in = nc.declare_dram_parameter("ident", [64, 64], bf16, isOutput=False)
    outT = nc.declare_dram_parameter("outT", [TQ, T], f32, isOutput=True)

    # asymmetric exchange halves: pairs {0,1,2} then {3} so the last
    # collective of a q-tile is as small as possible
    HALF_PAIRS = ((0, 1, 2), (3,))
    attn_dram = {
        (qt, hf): nc.dram_tensor(
            f"attn_d{qt}_{hf}", [128 * len(HALF_PAIRS[hf]), TQ], bf16
        )
        for qt in range(NT)
        for hf in range(2)
    }
    ag_out = {
        (qt, hf): nc.dram_tensor(
            f"ag_out{qt}_{hf}", [256 * len(HALF_PAIRS[hf]), TQ], bf16
        )
        for qt in range(NT)
        for hf in range(2)
    }
    PAIRS = [[0, 1], [2, 3], [4, 5], [6, 7]]

    with tile.TileContext(nc) as tc, ExitStack() as ctx:
        const_pool = ctx.enter_context(tc.tile_pool(name="const", bufs=1))
        qk_pool = ctx.enter_context(tc.tile_pool(name="qk", bufs=1))
        vn_pool = ctx.enter_context(tc.tile_pool(name="vn", bufs=1))
        stream_pool = ctx.enter_context(tc.tile_pool(name="stream", bufs=2))
        agt_pool = ctx.enter_context(tc.tile_pool(name="agt", bufs=4))
        v2t_pool = ctx.enter_context(tc.tile_pool(name="v2t", bufs=2))
        e_pool = ctx.enter_context(tc.tile_pool(name="etile", bufs=4))
        gath_pool = ctx.enter_context(tc.tile_pool(name="gath", bufs=2))
        attnall_pool = ctx.enter_context(tc.tile_pool(name="attnall", bufs=1))
        osb_pool = ctx.enter_context(tc.tile_pool(name="osb", bufs=2))
        proj_ps = ctx.enter_context(tc.tile_pool(name="projps", bufs=2, space="PSUM"))
        sc_ps = ctx.enter_context(tc.tile_pool(name="scps", bufs=2, space="PSUM"))
        av_ps = ctx.enter_context(tc.tile_pool(name="avps", bufs=2, space="PSUM"))

        # ---- weights (wq first + split: first proj matmuls start sooner) ----
        w_sb = {}
        for name, src in (("q", wq), ("k", wk), ("v", wv), ("o", wo)):
            t_ = const_pool.tile([128, NCC, TQ], bf16, tag=f"w{name}", name=f"w{name}")
            r = src.rearrange("(j p) d -> p j d", p=128)
            if name == "q":
                nc.sync.dma_start(out=t_[:, 0:2, :], in_=r[:, 0:2, :])
                nc.sync.dma_start(out=t_[:, 2:NCC, :], in_=r[:, 2:NCC, :])
            else:
                nc.sync.dma_start(out=t_[:], in_=r)
            w_sb[name] = t_
        band = const_pool.tile([128, 128], bf16)
        nc.sync.dma_start(out=band[:], in_=band_in[:])
        ident = const_pool.tile([128, 64], bf16)
        nc.sync.dma_start(out=ident[0:64, :], in_=ident_in[:])
        nc.sync.dma_start(out=ident[64:128, :], in_=ident_in[:])
        ones_t = const_pool.tile([128, 64], bf16)
        nc.vector.memset(ones_t[:], 1.0)

        # persistent tiles (one batch per core)
        q2t = [
            qk_pool.tile([128, T], bf16, tag=f"q2t{p}", name=f"q2t{p}")
            for p in range(NPAIR)
        ]
        k2t = [
            qk_pool.tile([128, T], bf16, tag=f"k2t{p}", name=f"k2t{p}")
            for p in range(NPAIR)
        ]
        vn = [
            vn_pool.tile([128, NKC, 65], bf16, tag=f"vn{h}", name=f"vn{h}")
            for h in range(2 * NPAIR)
        ]
        for h in range(2 * NPAIR):
            nc.vector.memset(vn[h][:, :, 64:65], 1.0)
        attnall = [
            attnall_pool.tile([128, T], bf16, tag=f"attnall{p}", name=f"attnall{p}")
            for p in range(NPAIR)
        ]

        xt_tiles = {}

        def fetch_x(tt):
            xt = stream_pool.tile([128, NCC, TQ], bf16, tag="xt", name="xt")
            r = xT[:, tt * TQ : (tt + 1) * TQ].rearrange("(j p) t -> p j t", p=128)
            if tt == 0:
                nc.gpsimd.dma_start(out=xt[:, 0:2, :], in_=r[:, 0:2, :])
                nc.gpsimd.dma_start(out=xt[:, 2:NCC, :], in_=r[:, 2:NCC, :])
            else:
                nc.gpsimd.dma_start(out=xt[:], in_=r)
            xt_tiles[tt] = xt

        v2t_tiles = {}

        def proj_chunk(tt, p, name):
            """One 128-col projection chunk: 8 accumulating matmuls.
            PSUM evacuation runs on the (otherwise idle) Pool engine."""
            xt = xt_tiles[tt]
            cs = slice(p * 128, (p + 1) * 128)
            col0 = tt * TQ
            ps = proj_ps.tile([128, TQ], f32, tag="proj", name="proj")
            for j in range(NCC):
                nc.tensor.matmul(
                    ps[:, :],
                    w_sb[name][:, j, cs],
                    xt[:, j, :],
                    start=(j == 0),
                    stop=(j == NCC - 1),
                )
            if name == "v":
                v2t = v2t_pool.tile([128, TQ], bf16, tag="v2t", name="v2t")
                nc.vector.tensor_copy(out=v2t[:], in_=ps[:, :])
                v2t_tiles[(tt, p)] = v2t
            else:
                dst = q2t[p] if name == "q" else k2t[p]
                nc.vector.tensor_copy(out=dst[:, col0 : col0 + TQ], in_=ps[:, :])

        def vtrans_chunk(tt, p):
            """Transpose V^T [64,128] slices -> VN [128,64] chunks (issued a
            little after proj-v so the Pool evacuation has completed)."""
            v2t = v2t_tiles.pop((tt, p))
            kc0 = tt * (TQ // KC)
            for h in range(2):
                pt = proj_ps.tile([128, TQ], f32, tag="proj", name="proj")
                for sc in range(TQ // KC):
                    nc.tensor.transpose(
                        pt[0:128, 32 * sc : 32 * sc + 32].bitcast(bf16),
                        v2t[64 * h : 64 * h + 64, sc * KC : (sc + 1) * KC],
                        ident[64 * h : 64 * h + 64, :],
                    )
                nc.vector.tensor_copy(
                    out=vn[2 * p + h][:, kc0 : kc0 + 4, 0:64],
                    in_=pt[0:128, 0:128]
                    .bitcast(bf16)
                    .rearrange("p (c d) -> p c d", d=64),
                )

        def attn_stream(p, qt, g_t):
            """Attention for q-tile qt, head pair p; yields at pipeline
            flush points so filler PE work can be interleaved. Leaves
            unnormalized attn in attnall[p], row-sums in g_t[h] slot 32p."""
            nk = (qt + 1) * (TQ // KC)
            qsl = slice(qt * TQ, (qt + 1) * TQ)
            groups = [list(range(g, min(g + GROUP, nk))) for g in range(0, nk, GROUP)]
            avp = {
                h: av_ps.tile([128, TQ], f32, tag="av", name="av") for h in range(2)
            }
            hs = slice(0, 64), slice(64, 128)
            pend = []

            def flush_one():
                h, g, ps = pend.pop(0)
                et = e_pool.tile([128, GROUP * TQ], bf16, tag="etile", name="etile")
                run = []

                def flush_run():
                    if not run:
                        return
                    j0, j1 = run[0], run[-1]
                    nc.scalar.activation(
                        out=et[:, j0 * TQ : (j1 + 1) * TQ],
                        in_=ps[:, j0 * TQ : (j1 + 1) * TQ],
                        func=mybir.ActivationFunctionType.Exp,
                        scale=0.125,
                    )
                    run.clear()

                for j, kc in enumerate(g):
                    o = kc * KC - qt * TQ
                    if o < 0:
                        run.append(j)
                        continue
                    flush_run()
                    nc.scalar.activation(
                        out=et[:, j * TQ + o : (j + 1) * TQ],
                        in_=ps[:, j * TQ + o : (j + 1) * TQ],
                        func=mybir.ActivationFunctionType.Exp,
                        scale=0.125,
                    )
                    nc.gpsimd.tensor_mul(
                        et[:, j * TQ + o : j * TQ + o + 128],
                        et[:, j * TQ + o : j * TQ + o + 128],
                        band[:],
                    )
                flush_run()
                for j, kc in enumerate(g):
                    o = max(0, kc * KC - qt * TQ)
                    nc.tensor.matmul(
                        avp[h][0:65, o:TQ],
                        vn[2 * p + h][:, kc, :],
                        et[:, j * TQ + o : (j + 1) * TQ],
                        start=(kc == 0),
                        stop=(kc == nk - 1),
                    )

            for g in groups:
                for h in range(2):
                    ps = sc_ps.tile([128, GROUP * TQ], f32, tag="sc", name="sc")
                    for j, kc in enumerate(g):
                        o = max(0, kc * KC - qt * TQ)
                        nc.tensor.matmul(
                            ps[:, j * TQ + o : (j + 1) * TQ],
                            k2t[p][hs[h], kc * KC : (kc + 1) * KC],
                            q2t[p][hs[h], qt * TQ + o : (qt + 1) * TQ],
                            start=True,
                            stop=True,
                        )
                    pend.append((h, g, ps))
                    while len(pend) > 2:
                        flush_one()
                        yield
            while pend:
                flush_one()
                yield

            # evacuate unnormalized attn + row-sums; free psum asap
            for h in range(2):
                nc.vector.tensor_copy(
                    out=g_t[h][32 * p : 32 * p + 1, :], in_=avp[h][64:65, :]
                )
                nc.vector.tensor_copy(
                    out=attnall[p][hs[h], qsl], in_=avp[h][0:64, :]
                )
            yield

        def norm_recip(g_t, gr_t, hf):
            """DVE part of normalization: reciprocal + bf16 cast."""
            # slots 0,32,64 (pairs 0-2) for hf=0; slot 96 (pair 3) for hf=1
            base, size = (0, 65) if hf == 0 else (96, 1)
            for h in range(2):
                nc.vector.reciprocal(
                    g_t[h][base : base + size, :], g_t[h][base : base + size, :]
                )
                gr_t[(h, hf)] = gath_pool.tile(
                    [128, TQ], bf16, tag=f"gatr{h}{hf}", name="gatr"
                )
                nc.vector.tensor_copy(
                    out=gr_t[(h, hf)][base : base + size, :],
                    in_=g_t[h][base : base + size, :],
                )

        def norm_apply(qt, gr_t, hf):
            """Normalize pairs of half hf of q-tile qt and stage their AG."""
            qsl = slice(qt * TQ, (qt + 1) * TQ)
            gr = {h: gr_t[(h, hf)] for h in range(2)}
            for idx, p in enumerate(HALF_PAIRS[hf]):
                slot = 32 * p
                rp = sc_ps.tile([128, GROUP * TQ], f32, tag="sc", name="sc")
                for h in range(2):
                    nc.tensor.matmul(
                        rp[64 * h : 64 * h + 64, 0:TQ],
                        ones_t[slot : slot + 1, :],
                        gr[h][slot : slot + 1, :],
                        start=True,
                        stop=True,
                        tile_position=(slot, 64 * h),
                    )
                rsb = gath_pool.tile([128, TQ], bf16, tag="rsb", name="rsb")
                nc.vector.tensor_copy(out=rsb[:, :], in_=rp[0:128, 0:TQ])
                nc.gpsimd.tensor_mul(
                    attnall[p][:, qsl], attnall[p][:, qsl], rsb[:, :]
                )
                nc.sync.dma_start(
                    out=attn_dram[(qt, hf)][idx * 128 : (idx + 1) * 128, :],
                    in_=attnall[p][:, qsl],
                )
            nrow = 128 * len(HALF_PAIRS[hf])
            if MODEL_NO_COLLECTIVE:
                nc.sync.dma_start(
                    out=ag_out[(qt, hf)][0:nrow, :], in_=attn_dram[(qt, hf)][:]
                )
                nc.sync.dma_start(
                    out=ag_out[(qt, hf)][nrow : 2 * nrow, :],
                    in_=attn_dram[(qt, hf)][:],
                )
            else:
                nc.gpsimd.collective_compute(
                    "AllGather",
                    mybir.AluOpType.bypass,
                    ins=[attn_dram[(qt, hf)][:]],
                    outs=[ag_out[(qt, hf)][:]],
                    replica_groups=PAIRS,
                )

        agt_tiles = {}

        def fetch_ag(qt, hf):
            nj = 2 * len(HALF_PAIRS[hf])
            agt = agt_pool.tile([128, nj, TQ], bf16, tag=f"agt{hf}", name="agt")
            nc.sync.dma_start(
                out=agt[:],
                in_=ag_out[(qt, hf)][:].rearrange("(j p) t -> p j t", p=128),
            )
            agt_tiles[(qt, hf)] = agt

        def out_chunk(qt, c4):
            """One 128-col out-projection chunk: 8 accumulating matmuls,
            first 6 contracting the pairs-{0,1,2} AG half, last 2 the rest."""
            cs = slice(c4 * 128, (c4 + 1) * 128)
            ps = proj_ps.tile([128, TQ], f32, tag="proj", name="proj")
            for j in range(NCC):
                hf = 0 if j < 6 else 1
                agt = agt_tiles[(qt, hf)]
                nc.tensor.matmul(
                    ps[:, :],
                    w_sb["o"][:, j, cs],
                    agt[:, j - 6 * hf, :],
                    start=(j == 0),
                    stop=(j == NCC - 1),
                )
            osb = osb_pool.tile([128, TQ], f32, tag="osb", name="osb")
            nc.vector.tensor_copy(out=osb[:], in_=ps[:, :])
            nc.sync.dma_start(
                out=outT[cs, qt * TQ : (qt + 1) * TQ], in_=osb[:]
            )

        # ---- schedule ----
        # Just-in-time interleave: pair p's exp-bound attention window hosts
        # the projection chunks of pair p+1 (same q-tile; pair 0 of the next
        # tile during pair 3) plus one out-projection chunk of qt-1, so the
        # PE stream stays dense and HAM keeps the tensor engine at full rate.
        fetch_x(0)
        for name in ("q", "k", "v"):
            proj_chunk(0, 0, name)
        vtrans_chunk(0, 0)

        for qt in range(NT):
            if qt < NT - 1:
                fetch_x(qt + 1)
            if qt >= 1:
                for hf in range(2):
                    fetch_ag(qt - 1, hf)
            g_t = {
                h: gath_pool.tile([128, TQ], f32, tag=f"gather{h}", name="gather")
                for h in range(2)
            }
            gr_t = {}
            for p in range(NPAIR):
                win = []
                tp = (qt, p + 1) if p < 3 else (qt + 1, 0)
                if tp[0] < NT:
                    win += [
                        (lambda t=tp[0], pp=tp[1], n=name: proj_chunk(t, pp, n))
                        for name in ("q", "k", "v")
                    ]
                    win.append(lambda t=tp[0], pp=tp[1]: vtrans_chunk(t, pp))
                if qt >= 1:
                    win.append(lambda q=qt - 1, c=p: out_chunk(q, c))
                rate = len(win) / ((qt + 1) * 4 + 1)
                fi = iter(win)
                acc = 0.0
                for k, _ in enumerate(attn_stream(p, qt, g_t)):
                    if p == 3 and k == 2 * qt + 3:
                        # pairs {0,1,2} normalization mid-window (their recip
                        # ran during earlier yields); AG-a triggers well
                        # before the tile ends
                        norm_apply(qt, gr_t, 0)
                    acc += rate
                    while acc >= 1.0:
                        acc -= 1.0
                        nxt = next(fi, None)
                        if nxt is not None:
                            nxt()
                if p == 2:
                    # pairs {0,1,2} recip runs on DVE during pair 3's window
                    norm_recip(g_t, gr_t, 0)
                    for nxt in fi:
                        nxt()
                elif p == 3:
                    norm_recip(g_t, gr_t, 1)
                    for nxt in fi:
                        nxt()
                    norm_apply(qt, gr_t, 1)
                else:
                    for nxt in fi:
                        nxt()
        for hf in range(2):
            fetch_ag(NT - 1, hf)
        for c4 in range(4):
            out_chunk(NT - 1, c4)

    _split_multi_waits(nc)
    return nc


def _prepare_inputs(x, W_qkv, W_out):
    import ml_dtypes

    bf16 = ml_dtypes.bfloat16
    band = (np.arange(128)[None, :] >= np.arange(128)[:, None]).astype(bf16)
    ident = np.eye(64, dtype=bf16)
    Wq = W_qkv[:, 0:D_MODEL]
    Wk = W_qkv[:, D_MODEL : 2 * D_MODEL]
    Wv = W_qkv[:, 2 * D_MODEL :]
    # out-proj contraction chunks 0..5 read the pairs-{0,1,2} AG half
    # (attn dims 0:384 from rank0, 512:896 from rank1), 6..7 the rest.
    row_perm = np.concatenate(
        [np.arange(0, 384), np.arange(512, 896), np.arange(384, 512), np.arange(896, 1024)]
    )
    Wo_p = W_out[row_perm, :]
    in_maps = []
    for c in range(N_CORES):
        b, hh = c // 2, c % 2
        hd = slice(512 * hh, 512 * (hh + 1))
        in_maps.append(
            {
                "xT": np.ascontiguousarray(x[b].T).astype(bf16),
                "wq": np.ascontiguousarray(Wq[:, hd]).astype(bf16),
                "wk": np.ascontiguousarray(Wk[:, hd]).astype(bf16),
                "wv": np.ascontiguousarray(Wv[:, hd]).astype(bf16),
                "wo": np.ascontiguousarray(Wo_p[:, hd]).astype(bf16),
                "band": band,
                "ident": ident,
            }
        )
    return in_maps


def run(x, W_qkv, W_out, trace=False):
    import sys

    if "/opt/trn_rl_repo" not in sys.path:
        sys.path.insert(0, "/opt/trn_rl_repo")
    from concourse.bass_utils import run_bass_kernel_spmd

    key = "program"
    if key not in _PROGRAM_CACHE:
        _PROGRAM_CACHE[key] = _build_program()
    nc = _PROGRAM_CACHE[key]
    in_maps = _prepare_inputs(x, W_qkv, W_out)
    res = run_bass_kernel_spmd(
        nc, in_maps, core_ids=list(range(N_CORES)), trace=trace
    )
    out = np.empty((B, T, D_MODEL), dtype=np.float32)
    for c in range(N_CORES):
        b, hh = c // 2, c % 2
        out[b, :, 512 * hh : 512 * (hh + 1)] = res.results[c]["outT"].T
    return out, res


def kernel(x, W_qkv, W_out):
    out, _ = run(
        np.asarray(x, dtype=np.float32),
        np.asarray(W_qkv, dtype=np.float32),
        np.asarray(W_out, dtype=np.float32),
    )
    return out


# revision 35
# speedup vs baseline: 1.0457x; 1.0457x over previous
"""Causal self-attention on 8 trn2 NeuronCores.

Sharding: (batch, head-half) per core. Core c handles batch b=c//2 and
heads hh*8..hh*8+7 where hh=c%2. QKV projection + attention run fully
local in bf16; the two cores of a batch exchange attention-output halves
with pair AllGathers (two per 512-token q-tile, staged per pair-half so
the exchange hides behind later compute); out-projection is
column-parallel within the pair (512 output cols/core); host assembles.

Schedule: the attention inner loop is scalar-engine (exp) bound, so QKV
projection chunks of the next t-tile and out-projection chunks of the
previous q-tile are interleaved between attention score/AV groups to
keep the PE stream dense (avoids HAM down-throttle).

Layout per core (pairs p=0..3, local heads 2p, 2p+1):
  Q2T/K2T[p] [128, T] bf16  transposed q/k head-dim-major.
  VN[head]   [128, 16, 65] bf16 V k-chunks + ones column so the AV
             matmul also emits the softmax row-sum at psum partition 64.
  scores     S^T chunk [128 k, <=512 q] f32 psum; diagonal chunks are
             computed sliced ([o:512]) instead of masked+memset.
  E^T        exp(S/8) bf16 via ACT; band-mask mult on diagonal block.
  attnall[p] [128, T] bf16 unnormalized; batched reciprocal + ones
             broadcast matmul normalize in place per pair-half.
W_out is row-permuted host-side so out-projection contraction chunks
0..3 come from the pairs-{0,1} AllGather and 4..7 from pairs-{2,3}.
"""

import numpy as np

D_MODEL = 1024
N_HEADS = 16
HEAD_DIM = 64
B = 4
T = 2048
N_CORES = 8
TQ = 512          # q tile
KC = 128          # k chunk
GROUP = 2         # k-chunks per exp group
NT = T // TQ      # q tiles per batch (4)
NKC = T // KC     # k chunks per batch (16)
NCC = D_MODEL // 128  # contraction chunks (8)
NPAIR = 4         # head pairs per core (8 heads)
MODEL_NO_COLLECTIVE = False  # timing-analysis only: swap AG for local DMA

_PROGRAM_CACHE = {}


def _split_multi_waits(nc, max_waits=1):
    """This toolchain's walrus encodes at most one sync-wait per
    instruction; hoist excess waits onto same-engine carrier nops."""
    import concourse.mybir as mybir

    ctr = 0
    for f in nc.m.functions:
        new_blocks = []
        for bb in f.blocks:
            insts = list(bb.instructions)
            if not any(
                inst.sync_info is not None and len(inst.sync_info.on_wait) > max_waits
                for inst in insts
            ):
                new_blocks.append(bb)
                continue
            out = []
            for inst in insts:
                si = inst.sync_info
                if si is not None and len(si.on_wait) > max_waits:
                    waits = list(si.on_wait)
                    excess = waits[max_waits:]
                    while excess:
                        ctr += 1
                        nop = mybir.InstNoOp(
                            name=f"waitcarrier-{ctr}", engine=inst.engine
                        )
                        nop.sync_info = mybir.SyncInfo(
                            on_wait=excess[:max_waits], on_update=[]
                        )
                        out.append(nop)
                        excess = excess[max_waits:]
                    si.on_wait = waits[:max_waits]
                out.append(inst)
            nb = mybir.BasicBlock(
                name=bb.name,
                instructions=[],
                IsPredicated=bb.IsPredicated,
                IsExit=bb.IsExit,
                IsLoopEntry=bb.IsLoopEntry,
            )
            for i in out:
                nb.add_instruction(i)
            new_blocks.append(nb)
        f.blocks = new_blocks


def _build_program():
    import concourse.bass as bass
    import concourse.mybir as mybir
    import concourse.tile as tile
    from contextlib import ExitStack

    f32 = mybir.dt.float32
    bf16 = mybir.dt.bfloat16
    nc = bass.Bass()

    xT = nc.declare_dram_parameter("xT", [D_MODEL, T], bf16, isOutput=False)
    wq = nc.declare_dram_parameter("wq", [D_MODEL, TQ], bf16, isOutput=False)
    wk = nc.declare_dram_parameter("wk", [D_MODEL, TQ], bf16, isOutput=False)
    wv = nc.declare_dram_parameter("wv", [D_MODEL, TQ], bf16, isOutput=False)
    wo = nc.declare_dram_parameter("wo", [D_MODEL, TQ], bf16, isOutput=False)
    band_in = nc.declare_dram_parameter("band", [128, 128], bf16, isOutput=False)
    ident_in = nc.declare_dram_parameter("ident", [64, 64], bf16, isOutput=False)
    outT = nc.declare_dram_parameter("outT", [TQ, T], f32, isOutput=True)

    # asymmetric exchange halves: pairs {0,1,2} then {3} so the last
    # collective of a q-tile is as small as possible
    HALF_PAIRS = ((0, 1, 2), (3,))
    attn_dram = {
        (qt, hf): nc.dram_tensor(
            f"attn_d{qt}_{hf}", [128 * len(HALF_PAIRS[hf]), TQ], bf16
        )
        for qt in range(NT)
        for hf in range(2)
    }
    ag_out = {
        (qt, hf): nc.dram_tensor(
            f"ag_out{qt}_{hf}", [256 * len(HALF_PAIRS[hf]), TQ], bf16
        )
        for qt in range(NT)
        for hf in range(2)
    }
    PAIRS = [[0, 1], [2, 3], [4, 5], [6, 7]]

    with tile.TileContext(nc) as tc, ExitStack() as ctx:
        const_pool = ctx.enter_context(tc.tile_pool(name="const", bufs=1))
        qk_pool = ctx.enter_context(tc.tile_pool(name="qk", bufs=1))
        vn_pool = ctx.enter_context(tc.tile_pool(name="vn", bufs=1))
        stream_pool = ctx.enter_context(tc.tile_pool(name="stream", bufs=2))
        agt_pool = ctx.enter_context(tc.tile_pool(name="agt", bufs=4))
        v2t_pool = ctx.enter_context(tc.tile_pool(name="v2t", bufs=2))
        e_pool = ctx.enter_context(tc.tile_pool(name="etile", bufs=4))
        gath_pool = ctx.enter_context(tc.tile_pool(name="gath", bufs=2))
        attnall_pool = ctx.enter_context(tc.tile_pool(name="attnall", bufs=1))
        osb_pool = ctx.enter_context(tc.tile_pool(name="osb", bufs=2))
        proj_ps = ctx.enter_context(tc.tile_pool(name="projps", bufs=2, space="PSUM"))
        sc_ps = ctx.enter_context(tc.tile_pool(name="scps", bufs=2, space="PSUM"))
        av_ps = ctx.enter_context(tc.tile_pool(name="avps", bufs=2, space="PSUM"))

        # ---- weights (wq first + split: first proj matmuls start sooner) ----
        w_sb = {}
        for name, src in (("q", wq), ("k", wk), ("v", wv), ("o", wo)):
            t_ = const_pool.tile([128, NCC, TQ], bf16, tag=f"w{name}", name=f"w{name}")
            r = src.rearrange("(j p) d -> p j d", p=128)
            if name == "q":
                nc.sync.dma_start(out=t_[:, 0:2, :], in_=r[:, 0:2, :])
                nc.sync.dma_start(out=t_[:, 2:NCC, :], in_=r[:, 2:NCC, :])
            else:
                nc.sync.dma_start(out=t_[:], in_=r)
            w_sb[name] = t_
        band = const_pool.tile([128, 128], bf16)
        nc.sync.dma_start(out=band[:], in_=band_in[:])
        ident = const_pool.tile([128, 64], bf16)
        nc.sync.dma_start(out=ident[0:64, :], in_=ident_in[:])
        nc.sync.dma_start(out=ident[64:128, :], in_=ident_in[:])
        ones_t = const_pool.tile([128, 64], bf16)
        nc.vector.memset(ones_t[:], 1.0)

        # persistent tiles (one batch per core)
        q2t = [
            qk_pool.tile([128, T], bf16, tag=f"q2t{p}", name=f"q2t{p}")
            for p in range(NPAIR)
        ]
        k2t = [
            qk_pool.tile([128, T], bf16, tag=f"k2t{p}", name=f"k2t{p}")
            for p in range(NPAIR)
        ]
        vn = [
            vn_pool.tile([128, NKC, 65], bf16, tag=f"vn{h}", name=f"vn{h}")
            for h in range(2 * NPAIR)
        ]
        for h in range(2 * NPAIR):
            nc.vector.memset(vn[h][:, :, 64:65], 1.0)
        attnall = [
            attnall_pool.tile([128, T], bf16, tag=f"attnall{p}", name=f"attnall{p}")
            for p in range(NPAIR)
        ]

        xt_tiles = {}

        def fetch_x(tt):
            xt = stream_pool.tile([128, NCC, TQ], bf16, tag="xt", name="xt")
            r = xT[:, tt * TQ : (tt + 1) * TQ].rearrange("(j p) t -> p j t", p=128)
            if tt == 0:
                nc.gpsimd.dma_start(out=xt[:, 0:2, :], in_=r[:, 0:2, :])
                nc.gpsimd.dma_start(out=xt[:, 2:NCC, :], in_=r[:, 2:NCC, :])
            else:
                nc.gpsimd.dma_start(out=xt[:], in_=r)
            xt_tiles[tt] = xt

        v2t_tiles = {}

        def proj_chunk(tt, p, name):
            """One 128-col projection chunk: 8 accumulating matmuls.
            PSUM evacuation runs on the (otherwise idle) Pool engine."""
            xt = xt_tiles[tt]
            cs = slice(p * 128, (p + 1) * 128)
            col0 = tt * TQ
            ps = proj_ps.tile([128, TQ], f32, tag="proj", name="proj")
            for j in range(NCC):
                nc.tensor.matmul(
                    ps[:, :],
                    w_sb[name][:, j, cs],
                    xt[:, j, :],
                    start=(j == 0),
                    stop=(j == NCC - 1),
                )
            if name == "v":
                v2t = v2t_pool.tile([128, TQ], bf16, tag="v2t", name="v2t")
                nc.vector.tensor_copy(out=v2t[:], in_=ps[:, :])
                v2t_tiles[(tt, p)] = v2t
            else:
                dst = q2t[p] if name == "q" else k2t[p]
                nc.vector.tensor_copy(out=dst[:, col0 : col0 + TQ], in_=ps[:, :])

        def vtrans_chunk(tt, p):
            """Transpose V^T [64,128] slices -> VN [128,64] chunks (issued a
            little after proj-v so the Pool evacuation has completed)."""
            v2t = v2t_tiles.pop((tt, p))
            kc0 = tt * (TQ // KC)
            for h in range(2):
                pt = proj_ps.tile([128, TQ], f32, tag="proj", name="proj")
                for sc in range(TQ // KC):
                    nc.tensor.transpose(
                        pt[0:128, 32 * sc : 32 * sc + 32].bitcast(bf16),
                        v2t[64 * h : 64 * h + 64, sc * KC : (sc + 1) * KC],
                        ident[64 * h : 64 * h + 64, :],
                    )
                nc.vector.tensor_copy(
                    out=vn[2 * p + h][:, kc0 : kc0 + 4, 0:64],
                    in_=pt[0:128, 0:128]
                    .bitcast(bf16)
                    .rearrange("p (c d) -> p c d", d=64),
                )

        def attn_stream(p, qt, g_t):
            """Attention for q-tile qt, head pair p; yields at pipeline
            flush points so filler PE work can be interleaved. Leaves
            unnormalized attn in attnall[p], row-sums in g_t[h] slot 32p."""
            nk = (qt + 1) * (TQ // KC)
            qsl = slice(qt * TQ, (qt + 1) * TQ)
            groups = [list(range(g, min(g + GROUP, nk))) for g in range(0, nk, GROUP)]
            avp = {
                h: av_ps.tile([128, TQ], f32, tag="av", name="av") for h in range(2)
            }
            hs = slice(0, 64), slice(64, 128)
            pend = []

            def flush_one():
                h, g, ps = pend.pop(0)
                et = e_pool.tile([128, GROUP * TQ], bf16, tag="etile", name="etile")
                run = []

                def flush_run():
                    if not run:
                        return
                    j0, j1 = run[0], run[-1]
                    nc.scalar.activation(
                        out=et[:, j0 * TQ : (j1 + 1) * TQ],
                        in_=ps[:, j0 * TQ : (j1 + 1) * TQ],
                        func=mybir.ActivationFunctionType.Exp,
                        scale=0.125,
                    )
                    run.clear()

                for j, kc in enumerate(g):
                    o = kc * KC - qt * TQ
                    if o < 0:
                        run.append(j)
                        continue
                    flush_run()
                    nc.scalar.activation(
                        out=et[:, j * TQ + o : (j + 1) * TQ],
                        in_=ps[:, j * TQ + o : (j + 1) * TQ],
                        func=mybir.ActivationFunctionType.Exp,
                        scale=0.125,
                    )
                    nc.vector.tensor_mul(
                        et[:, j * TQ + o : j * TQ + o + 128],
                        et[:, j * TQ + o : j * TQ + o + 128],
                        band[:],
                    )
                flush_run()
                for j, kc in enumerate(g):
                    o = max(0, kc * KC - qt * TQ)
                    nc.tensor.matmul(
                        avp[h][0:65, o:TQ],
                        vn[2 * p + h][:, kc, :],
                        et[:, j * TQ + o : (j + 1) * TQ],
                        start=(kc == 0),
                        stop=(kc == nk - 1),
                    )

            for g in groups:
                for h in range(2):
                    ps = sc_ps.tile([128, GROUP * TQ], f32, tag="sc", name="sc")
                    for j, kc in enumerate(g):
                        o = max(0, kc * KC - qt * TQ)
                        nc.tensor.matmul(
                            ps[:, j * TQ + o : (j + 1) * TQ],
                            k2t[p][hs[h], kc * KC : (kc + 1) * KC],
                            q2t[p][hs[h], qt * TQ + o : (qt + 1) * TQ],
                            start=True,
                            stop=True,
                        )
                    pend.append((h, g, ps))
                    while len(pend) > 2:
                        flush_one()
                        yield
            while pend:
                flush_one()
                yield

            # evacuate unnormalized attn + row-sums; free psum asap
            for h in range(2):
                nc.vector.tensor_copy(
                    out=g_t[h][32 * p : 32 * p + 1, :], in_=avp[h][64:65, :]
                )
                nc.vector.tensor_copy(
                    out=attnall[p][hs[h], qsl], in_=avp[h][0:64, :]
                )
            yield

        def norm_recip(g_t, gr_t, hf):
            """DVE part of normalization: reciprocal + bf16 cast."""
            # slots 0,32,64 (pairs 0-2) for hf=0; slot 96 (pair 3) for hf=1
            base, size = (0, 65) if hf == 0 else (96, 1)
            for h in range(2):
                nc.vector.reciprocal(
                    g_t[h][base : base + size, :], g_t[h][base : base + size, :]
                )
                gr_t[(h, hf)] = gath_pool.tile(
                    [128, TQ], bf16, tag=f"gatr{h}{hf}", name="gatr"
                )
                nc.vector.tensor_copy(
                    out=gr_t[(h, hf)][base : base + size, :],
                    in_=g_t[h][base : base + size, :],
                )

        def norm_apply(qt, gr_t, hf):
            """Normalize pairs of half hf of q-tile qt and stage their AG."""
            qsl = slice(qt * TQ, (qt + 1) * TQ)
            gr = {h: gr_t[(h, hf)] for h in range(2)}
            for idx, p in enumerate(HALF_PAIRS[hf]):
                slot = 32 * p
                rp = sc_ps.tile([128, GROUP * TQ], f32, tag="sc", name="sc")
                for h in range(2):
                    nc.tensor.matmul(
                        rp[64 * h : 64 * h + 64, 0:TQ],
                        ones_t[slot : slot + 1, :],
                        gr[h][slot : slot + 1, :],
                        start=True,
                        stop=True,
                        tile_position=(slot, 64 * h),
                    )
                rsb = gath_pool.tile([128, TQ], bf16, tag="rsb", name="rsb")
                nc.vector.tensor_copy(out=rsb[:, :], in_=rp[0:128, 0:TQ])
                nc.gpsimd.tensor_mul(
                    attnall[p][:, qsl], attnall[p][:, qsl], rsb[:, :]
                )
                nc.sync.dma_start(
                    out=attn_dram[(qt, hf)][idx * 128 : (idx + 1) * 128, :],
                    in_=attnall[p][:, qsl],
                )
            nrow = 128 * len(HALF_PAIRS[hf])
            if MODEL_NO_COLLECTIVE:
                nc.sync.dma_start(
                    out=ag_out[(qt, hf)][0:nrow, :], in_=attn_dram[(qt, hf)][:]
                )
                nc.sync.dma_start(
                    out=ag_out[(qt, hf)][nrow : 2 * nrow, :],
                    in_=attn_dram[(qt, hf)][:],
                )
            else:
                nc.gpsimd.collective_compute(
                    "AllGather",
                    mybir.AluOpType.bypass,
                    ins=[attn_dram[(qt, hf)][:]],
                    outs=[ag_out[(qt, hf)][:]],
                    replica_groups=PAIRS,
                )

        agt_tiles = {}

        def fetch_ag(qt, hf):
            nj = 2 * len(HALF_PAIRS[hf])
            agt = agt_pool.tile([128, nj, TQ], bf16, tag=f"agt{hf}", name="agt")
            nc.sync.dma_start(
                out=agt[:],
                in_=ag_out[(qt, hf)][:].rearrange("(j p) t -> p j t", p=128),
            )
            agt_tiles[(qt, hf)] = agt

        def out_chunk(qt, c4):
            """One 128-col out-projection chunk: 8 accumulating matmuls,
            first 6 contracting the pairs-{0,1,2} AG half, last 2 the rest."""
            cs = slice(c4 * 128, (c4 + 1) * 128)
            ps = proj_ps.tile([128, TQ], f32, tag="proj", name="proj")
            for j in range(NCC):
                hf = 0 if j < 6 else 1
                agt = agt_tiles[(qt, hf)]
                nc.tensor.matmul(
                    ps[:, :],
                    w_sb["o"][:, j, cs],
                    agt[:, j - 6 * hf, :],
                    start=(j == 0),
                    stop=(j == NCC - 1),
                )
            osb = osb_pool.tile([128, TQ], f32, tag="osb", name="osb")
            nc.vector.tensor_copy(out=osb[:], in_=ps[:, :])
            nc.sync.dma_start(
                out=outT[cs, qt * TQ : (qt + 1) * TQ], in_=osb[:]
            )

        # ---- schedule ----
        # Just-in-time interleave: pair p's exp-bound attention window hosts
        # the projection chunks of pair p+1 (same q-tile; pair 0 of the next
        # tile during pair 3) plus one out-projection chunk of qt-1, so the
        # PE stream stays dense and HAM keeps the tensor engine at full rate.
        fetch_x(0)
        for name in ("q", "k", "v"):
            proj_chunk(0, 0, name)
        vtrans_chunk(0, 0)

        for qt in range(NT):
            if qt < NT - 1:
                fetch_x(qt + 1)
            if qt >= 1:
                for hf in range(2):
                    fetch_ag(qt - 1, hf)
            g_t = {
                h: gath_pool.tile([128, TQ], f32, tag=f"gather{h}", name="gather")
                for h in range(2)
            }
            gr_t = {}
            for p in range(NPAIR):
                win = []
                tp = (qt, p + 1) if p < 3 else (qt + 1, 0)
                if tp[0] < NT:
                    win += [
                        (lambda t=tp[0], pp=tp[1], n=name: proj_chunk(t, pp, n))
                        for name in ("q", "k", "v")
                    ]
                    win.append(lambda t=tp[0], pp=tp[1]: vtrans_chunk(t, pp))
                if qt >= 1 and p >= 1:
                    # out-proj of qt-1: not in pair 0's window (its AG-b only
                    # lands a few us into this tile); pair 3 takes two chunks
                    win.append(lambda q=qt - 1, c=p - 1: out_chunk(q, c))
                    if p == 3:
                        win.append(lambda q=qt - 1: out_chunk(q, 3))
                rate = len(win) / ((qt + 1) * 4 + 1)
                fi = iter(win)
                acc = 0.0
                for k, _ in enumerate(attn_stream(p, qt, g_t)):
                    if p == 3 and k == 3 * qt + 4:
                        # pairs {0,1,2} normalization mid-window (their recip
                        # ran during earlier yields); AG-a triggers well
                        # before the tile ends
                        norm_apply(qt, gr_t, 0)
                    acc += rate
                    while acc >= 1.0:
                        acc -= 1.0
                        nxt = next(fi, None)
                        if nxt is not None:
                            nxt()
                if p == 2:
                    # pairs {0,1,2} recip runs on DVE during pair 3's window
                    norm_recip(g_t, gr_t, 0)
                    for nxt in fi:
                        nxt()
                elif p == 3:
                    norm_recip(g_t, gr_t, 1)
                    for nxt in fi:
                        nxt()
                    norm_apply(qt, gr_t, 1)
                else:
                    for nxt in fi:
                        nxt()
        for hf in range(2):
            fetch_ag(NT - 1, hf)
        for c4 in range(4):
            out_chunk(NT - 1, c4)

    _split_multi_waits(nc)
    return nc


def _prepare_inputs(x, W_qkv, W_out):
    import ml_dtypes

    bf16 = ml_dtypes.bfloat16
    band = (np.arange(128)[None, :] >= np.arange(128)[:, None]).astype(bf16)
    ident = np.eye(64, dtype=bf16)
    Wq = W_qkv[:, 0:D_MODEL]
    Wk = W_qkv[:, D_MODEL : 2 * D_MODEL]
    Wv = W_qkv[:, 2 * D_MODEL :]
    # out-proj contraction chunks 0..5 read the pairs-{0,1,2} AG half
    # (attn dims 0:384 from rank0, 512:896 from rank1), 6..7 the rest.
    row_perm = np.concatenate(
        [np.arange(0, 384), np.arange(512, 896), np.arange(384, 512), np.arange(896, 1024)]
    )
    Wo_p = W_out[row_perm, :]
    in_maps = []
    for c in range(N_CORES):
        b, hh = c // 2, c % 2
        hd = slice(512 * hh, 512 * (hh + 1))
        in_maps.append(
            {
                "xT": np.ascontiguousarray(x[b].T).astype(bf16),
                "wq": np.ascontiguousarray(Wq[:, hd]).astype(bf16),
                "wk": np.ascontiguousarray(Wk[:, hd]).astype(bf16),
                "wv": np.ascontiguousarray(Wv[:, hd]).astype(bf16),
                "wo": np.ascontiguousarray(Wo_p[:, hd]).astype(bf16),
                "band": band,
                "ident": ident,
            }
        )
    return in_maps


def run(x, W_qkv, W_out, trace=False):
    import sys

    if "/opt/trn_rl_repo" not in sys.path:
        sys.path.insert(0, "/opt/trn_rl_repo")
    from concourse.bass_utils import run_bass_kernel_spmd

    key = "program"
    if key not in _PROGRAM_CACHE:
        _PROGRAM_CACHE[key] = _build_program()
    nc = _PROGRAM_CACHE[key]
    in_maps = _prepare_inputs(x, W_qkv, W_out)
    res = run_bass_kernel_spmd(
        nc, in_maps, core_ids=list(range(N_CORES)), trace=trace
    )
    out = np.empty((B, T, D_MODEL), dtype=np.float32)
    for c in range(N_CORES):
        b, hh = c // 2, c % 2
        out[b, :, 512 * hh : 512 * (hh + 1)] = res.results[c]["outT"].T
    return out, res


def kernel(x, W_qkv, W_out):
    out, _ = run(
        np.asarray(x, dtype=np.float32),
        np.asarray(W_qkv, dtype=np.float32),
        np.asarray(W_out, dtype=np.float32),
    )
    return out


# revision 42
# speedup vs baseline: 1.1033x; 1.0551x over previous
"""Causal self-attention on 8 trn2 NeuronCores.

Sharding: (batch, head-half) per core. Core c handles batch b=c//2 and
heads hh*8..hh*8+7 where hh=c%2. QKV projection + attention run fully
local in bf16; the two cores of a batch exchange attention-output halves
with pair AllGathers (two per 512-token q-tile, staged per pair-half so
the exchange hides behind later compute); out-projection is
column-parallel within the pair (512 output cols/core); host assembles.

Schedule: the attention inner loop is scalar-engine (exp) bound, so QKV
projection chunks of the next t-tile and out-projection chunks of the
previous q-tile are interleaved between attention score/AV groups to
keep the PE stream dense (avoids HAM down-throttle).

Layout per core (pairs p=0..3, local heads 2p, 2p+1):
  Q2T/K2T[p] [128, T] bf16  transposed q/k head-dim-major.
  VN[head]   [128, 16, 65] bf16 V k-chunks + ones column so the AV
             matmul also emits the softmax row-sum at psum partition 64.
  scores     S^T chunk [128 k, <=512 q] f32 psum; diagonal chunks are
             computed sliced ([o:512]) instead of masked+memset.
  E^T        exp(S/8) bf16 via ACT; band-mask mult on diagonal block.
  attnall[p] [128, T] bf16 unnormalized; batched reciprocal + ones
             broadcast matmul normalize in place per pair-half.
W_out is row-permuted host-side so out-projection contraction chunks
0..3 come from the pairs-{0,1} AllGather and 4..7 from pairs-{2,3}.
"""

import numpy as np

D_MODEL = 1024
N_HEADS = 16
HEAD_DIM = 64
B = 4
T = 2048
N_CORES = 8
TQ = 512          # q tile
KC = 128          # k chunk
GROUP = 2         # k-chunks per exp group
NT = T // TQ      # q tiles per batch (4)
NKC = T // KC     # k chunks per batch (16)
NCC = D_MODEL // 128  # contraction chunks (8)
NPAIR = 4         # head pairs per core (8 heads)
MODEL_NO_COLLECTIVE = False  # timing-analysis only: swap AG for local DMA

_PROGRAM_CACHE = {}


def _split_multi_waits(nc, max_waits=1):
    """This toolchain's walrus encodes at most one sync-wait per
    instruction; hoist excess waits onto same-engine carrier nops."""
    import concourse.mybir as mybir

    ctr = 0
    for f in nc.m.functions:
        new_blocks = []
        for bb in f.blocks:
            insts = list(bb.instructions)
            if not any(
                inst.sync_info is not None and len(inst.sync_info.on_wait) > max_waits
                for inst in insts
            ):
                new_blocks.append(bb)
                continue
            out = []
            for inst in insts:
                si = inst.sync_info
                if si is not None and len(si.on_wait) > max_waits:
                    waits = list(si.on_wait)
                    excess = waits[max_waits:]
                    while excess:
                        ctr += 1
                        nop = mybir.InstNoOp(
                            name=f"waitcarrier-{ctr}", engine=inst.engine
                        )
                        nop.sync_info = mybir.SyncInfo(
                            on_wait=excess[:max_waits], on_update=[]
                        )
                        out.append(nop)
                        excess = excess[max_waits:]
                    si.on_wait = waits[:max_waits]
                out.append(inst)
            nb = mybir.BasicBlock(
                name=bb.name,
                instructions=[],
                IsPredicated=bb.IsPredicated,
                IsExit=bb.IsExit,
                IsLoopEntry=bb.IsLoopEntry,
            )
            for i in out:
                nb.add_instruction(i)
            new_blocks.append(nb)
        f.blocks = new_blocks


def _build_program():
    import concourse.bass as bass
    import concourse.mybir as mybir
    import concourse.tile as tile
    from contextlib import ExitStack

    f32 = mybir.dt.float32
    bf16 = mybir.dt.bfloat16
    nc = bass.Bass()

    xT = nc.declare_dram_parameter("xT", [D_MODEL, T], bf16, isOutput=False)
    wq = nc.declare_dram_parameter("wq", [D_MODEL, TQ], bf16, isOutput=False)
    wk = nc.declare_dram_parameter("wk", [D_MODEL, TQ], bf16, isOutput=False)
    wv = nc.declare_dram_parameter("wv", [D_MODEL, TQ], bf16, isOutput=False)
    wo = nc.declare_dram_parameter("wo", [D_MODEL, TQ], bf16, isOutput=False)
    band_in = nc.declare_dram_parameter("band", [128, 128], bf16, isOutput=False)
    ident_in = nc.declare_dram_parameter("ident", [64, 64], bf16, isOutput=False)
    outT = nc.declare_dram_parameter("outT", [TQ, T], f32, isOutput=True)

    # asymmetric exchange halves: pairs {0,1,2} then {3} so the last
    # collective of a q-tile is as small as possible
    HALF_PAIRS = ((0, 1, 2), (3,))
    attn_dram = {
        (qt, hf): nc.dram_tensor(
            f"attn_d{qt}_{hf}", [128 * len(HALF_PAIRS[hf]), TQ], bf16
        )
        for qt in range(NT)
        for hf in range(2)
    }
    ag_out = {
        (qt, hf): nc.dram_tensor(
            f"ag_out{qt}_{hf}", [256 * len(HALF_PAIRS[hf]), TQ], bf16
        )
        for qt in range(NT)
        for hf in range(2)
    }
    PAIRS = [[0, 1], [2, 3], [4, 5], [6, 7]]

    with tile.TileContext(nc) as tc, ExitStack() as ctx:
        const_pool = ctx.enter_context(tc.tile_pool(name="const", bufs=1))
        qk_pool = ctx.enter_context(tc.tile_pool(name="qk", bufs=1))
        vn_pool = ctx.enter_context(tc.tile_pool(name="vn", bufs=1))
        stream_pool = ctx.enter_context(tc.tile_pool(name="stream", bufs=2))
        agt_pool = ctx.enter_context(tc.tile_pool(name="agt", bufs=4))
        v2t_pool = ctx.enter_context(tc.tile_pool(name="v2t", bufs=2))
        e_pool = ctx.enter_context(tc.tile_pool(name="etile", bufs=4))
        gath_pool = ctx.enter_context(tc.tile_pool(name="gath", bufs=2))
        attnall_pool = ctx.enter_context(tc.tile_pool(name="attnall", bufs=1))
        osb_pool = ctx.enter_context(tc.tile_pool(name="osb", bufs=2))
        proj_ps = ctx.enter_context(tc.tile_pool(name="projps", bufs=2, space="PSUM"))
        sc_ps = ctx.enter_context(tc.tile_pool(name="scps", bufs=2, space="PSUM"))
        av_ps = ctx.enter_context(tc.tile_pool(name="avps", bufs=2, space="PSUM"))

        # ---- weights (wq first + split: first proj matmuls start sooner) ----
        w_sb = {}
        for name, src in (("q", wq), ("k", wk), ("v", wv), ("o", wo)):
            t_ = const_pool.tile([128, NCC, TQ], bf16, tag=f"w{name}", name=f"w{name}")
            r = src.rearrange("(j p) d -> p j d", p=128)
            if name == "q":
                nc.sync.dma_start(out=t_[:, 0:2, :], in_=r[:, 0:2, :])
                nc.sync.dma_start(out=t_[:, 2:NCC, :], in_=r[:, 2:NCC, :])
            else:
                nc.sync.dma_start(out=t_[:], in_=r)
            w_sb[name] = t_
        band = const_pool.tile([128, 128], bf16)
        nc.sync.dma_start(out=band[:], in_=band_in[:])
        ident = const_pool.tile([128, 64], bf16)
        nc.sync.dma_start(out=ident[0:64, :], in_=ident_in[:])
        nc.sync.dma_start(out=ident[64:128, :], in_=ident_in[:])
        ones_t = const_pool.tile([128, 64], bf16)
        nc.vector.memset(ones_t[:], 1.0)

        # persistent tiles (one batch per core)
        q2t = [
            qk_pool.tile([128, T], bf16, tag=f"q2t{p}", name=f"q2t{p}")
            for p in range(NPAIR)
        ]
        k2t = [
            qk_pool.tile([128, T], bf16, tag=f"k2t{p}", name=f"k2t{p}")
            for p in range(NPAIR)
        ]
        vn = [
            vn_pool.tile([128, NKC, 65], bf16, tag=f"vn{h}", name=f"vn{h}")
            for h in range(2 * NPAIR)
        ]
        for h in range(2 * NPAIR):
            nc.vector.memset(vn[h][:, :, 64:65], 1.0)
        attnall = [
            attnall_pool.tile([128, T], bf16, tag=f"attnall{p}", name=f"attnall{p}")
            for p in range(NPAIR)
        ]

        xt_tiles = {}

        def fetch_x(tt):
            xt = stream_pool.tile([128, NCC, TQ], bf16, tag="xt", name="xt")
            r = xT[:, tt * TQ : (tt + 1) * TQ].rearrange("(j p) t -> p j t", p=128)
            if tt == 0:
                nc.gpsimd.dma_start(out=xt[:, 0:2, :], in_=r[:, 0:2, :])
                nc.gpsimd.dma_start(out=xt[:, 2:NCC, :], in_=r[:, 2:NCC, :])
            else:
                nc.gpsimd.dma_start(out=xt[:], in_=r)
            xt_tiles[tt] = xt

        v2t_tiles = {}

        def proj_chunk(tt, p, name):
            """One 128-col projection chunk: 8 accumulating matmuls.
            PSUM evacuation runs on the (otherwise idle) Pool engine."""
            xt = xt_tiles[tt]
            cs = slice(p * 128, (p + 1) * 128)
            col0 = tt * TQ
            ps = proj_ps.tile([128, TQ], f32, tag="proj", name="proj")
            for j in range(NCC):
                nc.tensor.matmul(
                    ps[:, :],
                    w_sb[name][:, j, cs],
                    xt[:, j, :],
                    start=(j == 0),
                    stop=(j == NCC - 1),
                )
            if name == "v":
                v2t = v2t_pool.tile([128, TQ], bf16, tag="v2t", name="v2t")
                nc.vector.tensor_copy(out=v2t[:], in_=ps[:, :])
                v2t_tiles[(tt, p)] = v2t
            else:
                dst = q2t[p] if name == "q" else k2t[p]
                nc.vector.tensor_copy(out=dst[:, col0 : col0 + TQ], in_=ps[:, :])

        def vtrans_chunk(tt, p):
            """Transpose V^T [64,128] slices -> VN [128,64] chunks (issued a
            little after proj-v so the Pool evacuation has completed)."""
            v2t = v2t_tiles.pop((tt, p))
            kc0 = tt * (TQ // KC)
            for h in range(2):
                pt = proj_ps.tile([128, TQ], f32, tag="proj", name="proj")
                for sc in range(TQ // KC):
                    nc.tensor.transpose(
                        pt[0:128, 32 * sc : 32 * sc + 32].bitcast(bf16),
                        v2t[64 * h : 64 * h + 64, sc * KC : (sc + 1) * KC],
                        ident[64 * h : 64 * h + 64, :],
                    )
                nc.vector.tensor_copy(
                    out=vn[2 * p + h][:, kc0 : kc0 + 4, 0:64],
                    in_=pt[0:128, 0:128]
                    .bitcast(bf16)
                    .rearrange("p (c d) -> p c d", d=64),
                )

        def attn_stream(p, qt, g_t):
            """Attention for q-tile qt, head pair p; yields at pipeline
            flush points so filler PE work can be interleaved. Leaves
            unnormalized attn in attnall[p], row-sums in g_t[h] slot 32p."""
            nk = (qt + 1) * (TQ // KC)
            qsl = slice(qt * TQ, (qt + 1) * TQ)
            groups = [list(range(g, min(g + GROUP, nk))) for g in range(0, nk, GROUP)]
            avp = {
                h: av_ps.tile([128, TQ], f32, tag="av", name="av") for h in range(2)
            }
            hs = slice(0, 64), slice(64, 128)
            pend = []

            def flush_one():
                h, g, ps = pend.pop(0)
                et = e_pool.tile([128, GROUP * TQ], bf16, tag="etile", name="etile")
                run = []

                def flush_run():
                    if not run:
                        return
                    j0, j1 = run[0], run[-1]
                    nc.scalar.activation(
                        out=et[:, j0 * TQ : (j1 + 1) * TQ],
                        in_=ps[:, j0 * TQ : (j1 + 1) * TQ],
                        func=mybir.ActivationFunctionType.Exp,
                        scale=0.125,
                    )
                    run.clear()

                for j, kc in enumerate(g):
                    o = kc * KC - qt * TQ
                    if o < 0:
                        run.append(j)
                        continue
                    flush_run()
                    nc.scalar.activation(
                        out=et[:, j * TQ + o : (j + 1) * TQ],
                        in_=ps[:, j * TQ + o : (j + 1) * TQ],
                        func=mybir.ActivationFunctionType.Exp,
                        scale=0.125,
                    )
                    nc.vector.tensor_mul(
                        et[:, j * TQ + o : j * TQ + o + 128],
                        et[:, j * TQ + o : j * TQ + o + 128],
                        band[:],
                    )
                flush_run()
                for j, kc in enumerate(g):
                    o = max(0, kc * KC - qt * TQ)
                    nc.tensor.matmul(
                        avp[h][0:65, o:TQ],
                        vn[2 * p + h][:, kc, :],
                        et[:, j * TQ + o : (j + 1) * TQ],
                        start=(kc == 0),
                        stop=(kc == nk - 1),
                    )

            for g in groups:
                for h in range(2):
                    ps = sc_ps.tile([128, GROUP * TQ], f32, tag="sc", name="sc")
                    for j, kc in enumerate(g):
                        o = max(0, kc * KC - qt * TQ)
                        nc.tensor.matmul(
                            ps[:, j * TQ + o : (j + 1) * TQ],
                            k2t[p][hs[h], kc * KC : (kc + 1) * KC],
                            q2t[p][hs[h], qt * TQ + o : (qt + 1) * TQ],
                            start=True,
                            stop=True,
                        )
                    pend.append((h, g, ps))
                    while len(pend) > 2:
                        flush_one()
                        yield
            while pend:
                flush_one()
                yield

            # evacuate unnormalized attn + row-sums; free psum asap.
            # pair 3's two row-sums go to one tile (slots 64/96) so the
            # tile-end reciprocal is a single DVE op.
            for h in range(2):
                if p == 3:
                    nc.vector.tensor_copy(
                        out=g_t[3][64 + 32 * h : 65 + 32 * h, :],
                        in_=avp[h][64:65, :],
                    )
                else:
                    nc.vector.tensor_copy(
                        out=g_t[h][32 * p : 32 * p + 1, :], in_=avp[h][64:65, :]
                    )
                nc.vector.tensor_copy(
                    out=attnall[p][hs[h], qsl], in_=avp[h][0:64, :]
                )
            yield

        def norm_recip(g_t, gr_t, hf):
            """DVE part of normalization: reciprocal + bf16 cast."""
            if hf == 0:
                # slots 0,32,64 = pairs 0-2, per-head tiles
                for h in range(2):
                    nc.vector.reciprocal(g_t[h][0:65, :], g_t[h][0:65, :])
                    gr_t[(h, 0)] = gath_pool.tile(
                        [128, TQ], bf16, tag=f"gatr{h}0", name="gatr"
                    )
                    nc.vector.tensor_copy(
                        out=gr_t[(h, 0)][0:65, :], in_=g_t[h][0:65, :]
                    )
            else:
                # pair 3: both heads in one tile (slots 64/96) -> one recip
                nc.vector.reciprocal(g_t[3][64:97, :], g_t[3][64:97, :])
                for h in range(2):
                    gr_t[(h, 1)] = gath_pool.tile(
                        [128, TQ], bf16, tag=f"gatr{h}1", name="gatr"
                    )
                    nc.vector.tensor_copy(
                        out=gr_t[(h, 1)][96:97, :],
                        in_=g_t[3][64 + 32 * h : 65 + 32 * h, :],
                    )

        def norm_apply(qt, gr_t, hf):
            """Normalize pairs of half hf of q-tile qt and stage their AG."""
            qsl = slice(qt * TQ, (qt + 1) * TQ)
            gr = {h: gr_t[(h, hf)] for h in range(2)}
            for idx, p in enumerate(HALF_PAIRS[hf]):
                slot = 32 * p
                rp = sc_ps.tile([128, GROUP * TQ], f32, tag="sc", name="sc")
                for h in range(2):
                    nc.tensor.matmul(
                        rp[64 * h : 64 * h + 64, 0:TQ],
                        ones_t[slot : slot + 1, :],
                        gr[h][slot : slot + 1, :],
                        start=True,
                        stop=True,
                        tile_position=(slot, 64 * h),
                    )
                rsb = gath_pool.tile([128, TQ], bf16, tag="rsb", name="rsb")
                nc.vector.tensor_copy(out=rsb[:, :], in_=rp[0:128, 0:TQ])
                nc.gpsimd.tensor_mul(
                    attnall[p][:, qsl], attnall[p][:, qsl], rsb[:, :]
                )
                nc.sync.dma_start(
                    out=attn_dram[(qt, hf)][idx * 128 : (idx + 1) * 128, :],
                    in_=attnall[p][:, qsl],
                )
            nrow = 128 * len(HALF_PAIRS[hf])
            if MODEL_NO_COLLECTIVE:
                nc.sync.dma_start(
                    out=ag_out[(qt, hf)][0:nrow, :], in_=attn_dram[(qt, hf)][:]
                )
                nc.sync.dma_start(
                    out=ag_out[(qt, hf)][nrow : 2 * nrow, :],
                    in_=attn_dram[(qt, hf)][:],
                )
            else:
                nc.gpsimd.collective_compute(
                    "AllGather",
                    mybir.AluOpType.bypass,
                    ins=[attn_dram[(qt, hf)][:]],
                    outs=[ag_out[(qt, hf)][:]],
                    replica_groups=PAIRS,
                )
            # issue the read-back here: it must follow the collective in
            # program order for the RAW dependency to be tracked
            fetch_ag(qt, hf)

        agt_tiles = {}

        def fetch_ag(qt, hf):
            nj = 2 * len(HALF_PAIRS[hf])
            agt = agt_pool.tile([128, nj, TQ], bf16, tag=f"agt{hf}", name="agt")
            nc.sync.dma_start(
                out=agt[:],
                in_=ag_out[(qt, hf)][:].rearrange("(j p) t -> p j t", p=128),
            )
            agt_tiles[(qt, hf)] = agt

        def out_chunk(qt, c4):
            """One 128-col out-projection chunk: 8 accumulating matmuls,
            first 6 contracting the pairs-{0,1,2} AG half, last 2 the rest."""
            cs = slice(c4 * 128, (c4 + 1) * 128)
            ps = proj_ps.tile([128, TQ], f32, tag="proj", name="proj")
            for j in range(NCC):
                hf = 0 if j < 6 else 1
                agt = agt_tiles[(qt, hf)]
                nc.tensor.matmul(
                    ps[:, :],
                    w_sb["o"][:, j, cs],
                    agt[:, j - 6 * hf, :],
                    start=(j == 0),
                    stop=(j == NCC - 1),
                )
            osb = osb_pool.tile([128, TQ], f32, tag="osb", name="osb")
            nc.vector.tensor_copy(out=osb[:], in_=ps[:, :])
            nc.sync.dma_start(
                out=outT[cs, qt * TQ : (qt + 1) * TQ], in_=osb[:]
            )

        # ---- schedule ----
        # Just-in-time interleave: pair p's exp-bound attention window hosts
        # the projection chunks of pair p+1 (same q-tile; pair 0 of the next
        # tile during pair 3) plus one out-projection chunk of qt-1, so the
        # PE stream stays dense and HAM keeps the tensor engine at full rate.
        fetch_x(0)
        for name in ("q", "k", "v"):
            proj_chunk(0, 0, name)
        vtrans_chunk(0, 0)

        pending_apply_b = None
        for qt in range(NT):
            if qt < NT - 1:
                fetch_x(qt + 1)
            g_t = {
                h: gath_pool.tile([128, TQ], f32, tag=f"gather{h}", name="gather")
                for h in (0, 1, 3)
            }
            gr_t = {}
            for p in range(NPAIR):
                win = []
                tp = (qt, p + 1) if p < 3 else (qt + 1, 0)
                if tp[0] < NT:
                    win += [
                        (lambda t=tp[0], pp=tp[1], n=name: proj_chunk(t, pp, n))
                        for name in ("q", "k", "v")
                    ]
                if qt >= 1 and p >= 1:
                    # out-proj of qt-1: not in pair 0's window (its AG-b only
                    # lands a few us into this tile); pair 3 takes two chunks
                    win.append(lambda q=qt - 1, c=p - 1: out_chunk(q, c))
                    if p == 3:
                        win.append(lambda q=qt - 1: out_chunk(q, 3))
                if tp[0] < NT:
                    # vtrans last: needed only by pair p+1's diagonal groups
                    win.append(lambda t=tp[0], pp=tp[1]: vtrans_chunk(t, pp))
                if p == 0 and pending_apply_b is not None:
                    # previous tile's pair-3 normalization + AG: its recip
                    # was issued at tile end and has finished by now
                    win.insert(min(1, len(win)), pending_apply_b)
                    pending_apply_b = None
                rate = len(win) / ((qt + 1) * 4 + 1)
                fi = iter(win)
                acc = 0.0
                for k, _ in enumerate(attn_stream(p, qt, g_t)):
                    if p == 3 and k == 3 * qt + 4:
                        # pairs {0,1,2} normalization mid-window (their recip
                        # ran during earlier yields); AG-a triggers well
                        # before the tile ends
                        norm_apply(qt, gr_t, 0)
                    acc += rate
                    while acc >= 1.0:
                        acc -= 1.0
                        nxt = next(fi, None)
                        if nxt is not None:
                            nxt()
                if p == 2:
                    # pairs {0,1,2} recip runs on DVE during pair 3's window
                    norm_recip(g_t, gr_t, 0)
                elif p == 3:
                    norm_recip(g_t, gr_t, 1)
                for nxt in fi:
                    nxt()
                if p == 3:
                    if qt < NT - 1:
                        pending_apply_b = (
                            lambda q=qt, g=gr_t: norm_apply(q, g, 1)
                        )
                    else:
                        norm_apply(qt, gr_t, 1)
        # tail: contract the big AG half for every column chunk first, so
        # only the 2 pair-3 contraction matmuls per chunk wait on AG-b
        tail_ps = {}
        for c4 in range(4):
            cs = slice(c4 * 128, (c4 + 1) * 128)
            pool, w = (proj_ps, TQ) if c4 < 2 else (sc_ps, GROUP * TQ)
            ps = pool.tile([128, w], f32, tag="proj" if c4 < 2 else "sc", name="tail")
            tail_ps[c4] = ps
            agt = agt_tiles[(NT - 1, 0)]
            for j in range(6):
                nc.tensor.matmul(
                    ps[:, 0:TQ],
                    w_sb["o"][:, j, cs],
                    agt[:, j, :],
                    start=(j == 0),
                    stop=False,
                )
        for c4 in range(4):
            cs = slice(c4 * 128, (c4 + 1) * 128)
            ps = tail_ps[c4]
            agt = agt_tiles[(NT - 1, 1)]
            for j in range(6, NCC):
                nc.tensor.matmul(
                    ps[:, 0:TQ],
                    w_sb["o"][:, j, cs],
                    agt[:, j - 6, :],
                    start=False,
                    stop=(j == NCC - 1),
                )
            osb = osb_pool.tile([128, TQ], f32, tag="osb", name="osb")
            nc.vector.tensor_copy(out=osb[:], in_=ps[:, 0:TQ])
            nc.sync.dma_start(
                out=outT[cs, (NT - 1) * TQ : NT * TQ], in_=osb[:]
            )

    _split_multi_waits(nc)
    return nc


def _prepare_inputs(x, W_qkv, W_out):
    import ml_dtypes

    bf16 = ml_dtypes.bfloat16
    band = (np.arange(128)[None, :] >= np.arange(128)[:, None]).astype(bf16)
    ident = np.eye(64, dtype=bf16)
    Wq = W_qkv[:, 0:D_MODEL]
    Wk = W_qkv[:, D_MODEL : 2 * D_MODEL]
    Wv = W_qkv[:, 2 * D_MODEL :]
    # out-proj contraction chunks 0..5 read the pairs-{0,1,2} AG half
    # (attn dims 0:384 from rank0, 512:896 from rank1), 6..7 the rest.
    row_perm = np.concatenate(
        [np.arange(0, 384), np.arange(512, 896), np.arange(384, 512), np.arange(896, 1024)]
    )
    Wo_p = W_out[row_perm, :]
    in_maps = []
    for c in range(N_CORES):
        b, hh = c // 2, c % 2
        hd = slice(512 * hh, 512 * (hh + 1))
        in_maps.append(
            {
                "xT": np.ascontiguousarray(x[b].T).astype(bf16),
                "wq": np.ascontiguousarray(Wq[:, hd]).astype(bf16),
                "wk": np.ascontiguousarray(Wk[:, hd]).astype(bf16),
                "wv": np.ascontiguousarray(Wv[:, hd]).astype(bf16),
                "wo": np.ascontiguousarray(Wo_p[:, hd]).astype(bf16),
                "band": band,
                "ident": ident,
            }
        )
    return in_maps


def run(x, W_qkv, W_out, trace=False):
    import sys

    if "/opt/trn_rl_repo" not in sys.path:
        sys.path.insert(0, "/opt/trn_rl_repo")
    from concourse.bass_utils import run_bass_kernel_spmd

    key = "program"
    if key not in _PROGRAM_CACHE:
        _PROGRAM_CACHE[key] = _build_program()
    nc = _PROGRAM_CACHE[key]
    in_maps = _prepare_inputs(x, W_qkv, W_out)
    res = run_bass_kernel_spmd(
        nc, in_maps, core_ids=list(range(N_CORES)), trace=trace
    )
    out = np.empty((B, T, D_MODEL), dtype=np.float32)
    for c in range(N_CORES):
        b, hh = c // 2, c % 2
        out[b, :, 512 * hh : 512 * (hh + 1)] = res.results[c]["outT"].T
    return out, res


def kernel(x, W_qkv, W_out):
    out, _ = run(
        np.asarray(x, dtype=np.float32),
        np.asarray(W_qkv, dtype=np.float32),
        np.asarray(W_out, dtype=np.float32),
    )
    return out


# revision 43
# speedup vs baseline: 1.1070x; 1.0033x over previous
"""Causal self-attention on 8 trn2 NeuronCores.

Sharding: (batch, head-half) per core. Core c handles batch b=c//2 and
heads hh*8..hh*8+7 where hh=c%2. QKV projection + attention run fully
local in bf16; the two cores of a batch exchange attention-output halves
with pair AllGathers (two per 512-token q-tile, staged per pair-half so
the exchange hides behind later compute); out-projection is
column-parallel within the pair (512 output cols/core); host assembles.

Schedule: the attention inner loop is scalar-engine (exp) bound, so QKV
projection chunks of the next t-tile and out-projection chunks of the
previous q-tile are interleaved between attention score/AV groups to
keep the PE stream dense (avoids HAM down-throttle).

Layout per core (pairs p=0..3, local heads 2p, 2p+1):
  Q2T/K2T[p] [128, T] bf16  transposed q/k head-dim-major.
  VN[head]   [128, 16, 65] bf16 V k-chunks + ones column so the AV
             matmul also emits the softmax row-sum at psum partition 64.
  scores     S^T chunk [128 k, <=512 q] f32 psum; diagonal chunks are
             computed sliced ([o:512]) instead of masked+memset.
  E^T        exp(S/8) bf16 via ACT; band-mask mult on diagonal block.
  attnall[p] [128, T] bf16 unnormalized; batched reciprocal + ones
             broadcast matmul normalize in place per pair-half.
W_out is row-permuted host-side so out-projection contraction chunks
0..3 come from the pairs-{0,1} AllGather and 4..7 from pairs-{2,3}.
"""

import numpy as np

D_MODEL = 1024
N_HEADS = 16
HEAD_DIM = 64
B = 4
T = 2048
N_CORES = 8
TQ = 512          # q tile
KC = 128          # k chunk
GROUP = 2         # k-chunks per exp group
NT = T // TQ      # q tiles per batch (4)
NKC = T // KC     # k chunks per batch (16)
NCC = D_MODEL // 128  # contraction chunks (8)
NPAIR = 4         # head pairs per core (8 heads)
MODEL_NO_COLLECTIVE = False  # timing-analysis only: swap AG for local DMA

_PROGRAM_CACHE = {}


def _split_multi_waits(nc, max_waits=1):
    """This toolchain's walrus encodes at most one sync-wait per
    instruction; hoist excess waits onto same-engine carrier nops."""
    import concourse.mybir as mybir

    ctr = 0
    for f in nc.m.functions:
        new_blocks = []
        for bb in f.blocks:
            insts = list(bb.instructions)
            if not any(
                inst.sync_info is not None and len(inst.sync_info.on_wait) > max_waits
                for inst in insts
            ):
                new_blocks.append(bb)
                continue
            out = []
            for inst in insts:
                si = inst.sync_info
                if si is not None and len(si.on_wait) > max_waits:
                    waits = list(si.on_wait)
                    excess = waits[max_waits:]
                    while excess:
                        ctr += 1
                        nop = mybir.InstNoOp(
                            name=f"waitcarrier-{ctr}", engine=inst.engine
                        )
                        nop.sync_info = mybir.SyncInfo(
                            on_wait=excess[:max_waits], on_update=[]
                        )
                        out.append(nop)
                        excess = excess[max_waits:]
                    si.on_wait = waits[:max_waits]
                out.append(inst)
            nb = mybir.BasicBlock(
                name=bb.name,
                instructions=[],
                IsPredicated=bb.IsPredicated,
                IsExit=bb.IsExit,
                IsLoopEntry=bb.IsLoopEntry,
            )
            for i in out:
                nb.add_instruction(i)
            new_blocks.append(nb)
        f.blocks = new_blocks


def _build_program():
    import concourse.bass as bass
    import concourse.mybir as mybir
    import concourse.tile as tile
    from contextlib import ExitStack

    f32 = mybir.dt.float32
    bf16 = mybir.dt.bfloat16
    nc = bass.Bass()

    xT = nc.declare_dram_parameter("xT", [D_MODEL, T], bf16, isOutput=False)
    wq = nc.declare_dram_parameter("wq", [D_MODEL, TQ], bf16, isOutput=False)
    wk = nc.declare_dram_parameter("wk", [D_MODEL, TQ], bf16, isOutput=False)
    wv = nc.declare_dram_parameter("wv", [D_MODEL, TQ], bf16, isOutput=False)
    wo = nc.declare_dram_parameter("wo", [D_MODEL, TQ], bf16, isOutput=False)
    band_in = nc.declare_dram_parameter("band", [128, 128], bf16, isOutput=False)
    ident_in = nc.declare_dram_parameter("ident", [64, 64], bf16, isOutput=False)
    outT = nc.declare_dram_parameter("outT", [TQ, T], f32, isOutput=True)

    # asymmetric exchange halves: pairs {0,1,2} then {3} so the last
    # collective of a q-tile is as small as possible
    HALF_PAIRS = ((0, 1, 2), (3,))
    attn_dram = {
        (qt, hf): nc.dram_tensor(
            f"attn_d{qt}_{hf}", [128 * len(HALF_PAIRS[hf]), TQ], bf16
        )
        for qt in range(NT)
        for hf in range(2)
    }
    ag_out = {
        (qt, hf): nc.dram_tensor(
            f"ag_out{qt}_{hf}", [256 * len(HALF_PAIRS[hf]), TQ], bf16
        )
        for qt in range(NT)
        for hf in range(2)
    }
    PAIRS = [[0, 1], [2, 3], [4, 5], [6, 7]]

    with tile.TileContext(nc) as tc, ExitStack() as ctx:
        const_pool = ctx.enter_context(tc.tile_pool(name="const", bufs=1))
        qk_pool = ctx.enter_context(tc.tile_pool(name="qk", bufs=1))
        vn_pool = ctx.enter_context(tc.tile_pool(name="vn", bufs=1))
        stream_pool = ctx.enter_context(tc.tile_pool(name="stream", bufs=2))
        agt_pool = ctx.enter_context(tc.tile_pool(name="agt", bufs=4))
        v2t_pool = ctx.enter_context(tc.tile_pool(name="v2t", bufs=2))
        e_pool = ctx.enter_context(tc.tile_pool(name="etile", bufs=4))
        gath_pool = ctx.enter_context(tc.tile_pool(name="gath", bufs=2))
        attnall_pool = ctx.enter_context(tc.tile_pool(name="attnall", bufs=1))
        osb_pool = ctx.enter_context(tc.tile_pool(name="osb", bufs=2))
        proj_ps = ctx.enter_context(tc.tile_pool(name="projps", bufs=2, space="PSUM"))
        sc_ps = ctx.enter_context(tc.tile_pool(name="scps", bufs=2, space="PSUM"))
        av_ps = ctx.enter_context(tc.tile_pool(name="avps", bufs=2, space="PSUM"))

        # ---- weights (wq first + split: first proj matmuls start sooner) ----
        w_sb = {}
        for name, src in (("q", wq), ("k", wk), ("v", wv), ("o", wo)):
            t_ = const_pool.tile([128, NCC, TQ], bf16, tag=f"w{name}", name=f"w{name}")
            r = src.rearrange("(j p) d -> p j d", p=128)
            if name == "q":
                nc.sync.dma_start(out=t_[:, 0:2, :], in_=r[:, 0:2, :])
                nc.sync.dma_start(out=t_[:, 2:NCC, :], in_=r[:, 2:NCC, :])
            else:
                nc.sync.dma_start(out=t_[:], in_=r)
            w_sb[name] = t_
        band = const_pool.tile([128, 128], bf16)
        nc.sync.dma_start(out=band[:], in_=band_in[:])
        ident = const_pool.tile([128, 64], bf16)
        nc.sync.dma_start(out=ident[0:64, :], in_=ident_in[:])
        nc.sync.dma_start(out=ident[64:128, :], in_=ident_in[:])
        ones_t = const_pool.tile([128, 64], bf16)
        nc.vector.memset(ones_t[:], 1.0)

        # persistent tiles (one batch per core)
        q2t = [
            qk_pool.tile([128, T], bf16, tag=f"q2t{p}", name=f"q2t{p}")
            for p in range(NPAIR)
        ]
        k2t = [
            qk_pool.tile([128, T], bf16, tag=f"k2t{p}", name=f"k2t{p}")
            for p in range(NPAIR)
        ]
        vn = [
            vn_pool.tile([128, NKC, 65], bf16, tag=f"vn{h}", name=f"vn{h}")
            for h in range(2 * NPAIR)
        ]
        for h in range(2 * NPAIR):
            nc.vector.memset(vn[h][:, :, 64:65], 1.0)
        attnall = [
            attnall_pool.tile([128, T], bf16, tag=f"attnall{p}", name=f"attnall{p}")
            for p in range(NPAIR)
        ]

        xt_tiles = {}

        def fetch_x(tt):
            xt = stream_pool.tile([128, NCC, TQ], bf16, tag="xt", name="xt")
            r = xT[:, tt * TQ : (tt + 1) * TQ].rearrange("(j p) t -> p j t", p=128)
            if tt == 0:
                nc.gpsimd.dma_start(out=xt[:, 0:2, :], in_=r[:, 0:2, :])
                nc.gpsimd.dma_start(out=xt[:, 2:NCC, :], in_=r[:, 2:NCC, :])
            else:
                nc.gpsimd.dma_start(out=xt[:], in_=r)
            xt_tiles[tt] = xt

        v2t_tiles = {}

        def proj_chunk(tt, p, name):
            """One 128-col projection chunk: 8 accumulating matmuls.
            PSUM evacuation runs on the (otherwise idle) Pool engine."""
            xt = xt_tiles[tt]
            cs = slice(p * 128, (p + 1) * 128)
            col0 = tt * TQ
            ps = proj_ps.tile([128, TQ], f32, tag="proj", name="proj")
            for j in range(NCC):
                nc.tensor.matmul(
                    ps[:, :],
                    w_sb[name][:, j, cs],
                    xt[:, j, :],
                    start=(j == 0),
                    stop=(j == NCC - 1),
                )
            if name == "v":
                v2t = v2t_pool.tile([128, TQ], bf16, tag="v2t", name="v2t")
                nc.vector.tensor_copy(out=v2t[:], in_=ps[:, :])
                v2t_tiles[(tt, p)] = v2t
            else:
                dst = q2t[p] if name == "q" else k2t[p]
                nc.vector.tensor_copy(out=dst[:, col0 : col0 + TQ], in_=ps[:, :])

        def vtrans_chunk(tt, p):
            """Transpose V^T [64,128] slices -> VN [128,64] chunks (issued a
            little after proj-v so the Pool evacuation has completed)."""
            v2t = v2t_tiles.pop((tt, p))
            kc0 = tt * (TQ // KC)
            for h in range(2):
                pt = proj_ps.tile([128, TQ], f32, tag="proj", name="proj")
                for sc in range(TQ // KC):
                    nc.tensor.transpose(
                        pt[0:128, 32 * sc : 32 * sc + 32].bitcast(bf16),
                        v2t[64 * h : 64 * h + 64, sc * KC : (sc + 1) * KC],
                        ident[64 * h : 64 * h + 64, :],
                    )
                nc.vector.tensor_copy(
                    out=vn[2 * p + h][:, kc0 : kc0 + 4, 0:64],
                    in_=pt[0:128, 0:128]
                    .bitcast(bf16)
                    .rearrange("p (c d) -> p c d", d=64),
                )

        def attn_stream(p, qt, g_t):
            """Attention for q-tile qt, head pair p; yields at pipeline
            flush points so filler PE work can be interleaved. Leaves
            unnormalized attn in attnall[p], row-sums in g_t[h] slot 32p."""
            nk = (qt + 1) * (TQ // KC)
            qsl = slice(qt * TQ, (qt + 1) * TQ)
            groups = [list(range(g, min(g + GROUP, nk))) for g in range(0, nk, GROUP)]
            avp = {
                h: av_ps.tile([128, TQ], f32, tag="av", name="av") for h in range(2)
            }
            hs = slice(0, 64), slice(64, 128)
            pend = []

            def flush_one():
                h, g, ps = pend.pop(0)
                et = e_pool.tile([128, GROUP * TQ], bf16, tag="etile", name="etile")
                run = []

                def flush_run():
                    if not run:
                        return
                    j0, j1 = run[0], run[-1]
                    nc.scalar.activation(
                        out=et[:, j0 * TQ : (j1 + 1) * TQ],
                        in_=ps[:, j0 * TQ : (j1 + 1) * TQ],
                        func=mybir.ActivationFunctionType.Exp,
                        scale=0.125,
                    )
                    run.clear()

                for j, kc in enumerate(g):
                    o = kc * KC - qt * TQ
                    if o < 0:
                        run.append(j)
                        continue
                    flush_run()
                    nc.scalar.activation(
                        out=et[:, j * TQ + o : (j + 1) * TQ],
                        in_=ps[:, j * TQ + o : (j + 1) * TQ],
                        func=mybir.ActivationFunctionType.Exp,
                        scale=0.125,
                    )
                    nc.vector.tensor_mul(
                        et[:, j * TQ + o : j * TQ + o + 128],
                        et[:, j * TQ + o : j * TQ + o + 128],
                        band[:],
                    )
                flush_run()
                for j, kc in enumerate(g):
                    o = max(0, kc * KC - qt * TQ)
                    nc.tensor.matmul(
                        avp[h][0:65, o:TQ],
                        vn[2 * p + h][:, kc, :],
                        et[:, j * TQ + o : (j + 1) * TQ],
                        start=(kc == 0),
                        stop=(kc == nk - 1),
                    )

            for g in groups:
                for h in range(2):
                    ps = sc_ps.tile([128, GROUP * TQ], f32, tag="sc", name="sc")
                    for j, kc in enumerate(g):
                        o = max(0, kc * KC - qt * TQ)
                        nc.tensor.matmul(
                            ps[:, j * TQ + o : (j + 1) * TQ],
                            k2t[p][hs[h], kc * KC : (kc + 1) * KC],
                            q2t[p][hs[h], qt * TQ + o : (qt + 1) * TQ],
                            start=True,
                            stop=True,
                        )
                    pend.append((h, g, ps))
                    while len(pend) > 2:
                        flush_one()
                        yield
            while pend:
                flush_one()
                yield

            # evacuate unnormalized attn + row-sums; free psum asap.
            # pair 3's two row-sums go to one tile (slots 64/96) so the
            # tile-end reciprocal is a single DVE op.
            for h in range(2):
                if p == 3:
                    nc.vector.tensor_copy(
                        out=g_t[3][64 + 32 * h : 65 + 32 * h, :],
                        in_=avp[h][64:65, :],
                    )
                else:
                    nc.vector.tensor_copy(
                        out=g_t[h][32 * p : 32 * p + 1, :], in_=avp[h][64:65, :]
                    )
                nc.vector.tensor_copy(
                    out=attnall[p][hs[h], qsl], in_=avp[h][0:64, :]
                )
            yield

        def norm_recip(g_t, gr_t, hf):
            """DVE part of normalization: reciprocal + bf16 cast."""
            if hf == 0:
                # slots 0,32,64 = pairs 0-2, per-head tiles
                for h in range(2):
                    nc.vector.reciprocal(g_t[h][0:65, :], g_t[h][0:65, :])
                    gr_t[(h, 0)] = gath_pool.tile(
                        [128, TQ], bf16, tag=f"gatr{h}0", name="gatr"
                    )
                    nc.vector.tensor_copy(
                        out=gr_t[(h, 0)][0:65, :], in_=g_t[h][0:65, :]
                    )
            else:
                # pair 3: both heads in one tile (slots 64/96) -> one recip
                nc.vector.reciprocal(g_t[3][64:97, :], g_t[3][64:97, :])
                for h in range(2):
                    gr_t[(h, 1)] = gath_pool.tile(
                        [128, TQ], bf16, tag=f"gatr{h}1", name="gatr"
                    )
                    nc.vector.tensor_copy(
                        out=gr_t[(h, 1)][96:97, :],
                        in_=g_t[3][64 + 32 * h : 65 + 32 * h, :],
                    )

        def norm_apply(qt, gr_t, hf):
            """Normalize pairs of half hf of q-tile qt and stage their AG."""
            qsl = slice(qt * TQ, (qt + 1) * TQ)
            gr = {h: gr_t[(h, hf)] for h in range(2)}
            for idx, p in enumerate(HALF_PAIRS[hf]):
                slot = 32 * p
                rp = sc_ps.tile([128, GROUP * TQ], f32, tag="sc", name="sc")
                for h in range(2):
                    nc.tensor.matmul(
                        rp[64 * h : 64 * h + 64, 0:TQ],
                        ones_t[slot : slot + 1, :],
                        gr[h][slot : slot + 1, :],
                        start=True,
                        stop=True,
                        tile_position=(slot, 64 * h),
                    )
                rsb = gath_pool.tile([128, TQ], bf16, tag="rsb", name="rsb")
                nc.vector.tensor_copy(out=rsb[:, :], in_=rp[0:128, 0:TQ])
                nc.gpsimd.tensor_mul(
                    attnall[p][:, qsl], attnall[p][:, qsl], rsb[:, :]
                )
                nc.sync.dma_start(
                    out=attn_dram[(qt, hf)][idx * 128 : (idx + 1) * 128, :],
                    in_=attnall[p][:, qsl],
                )
            nrow = 128 * len(HALF_PAIRS[hf])
            if MODEL_NO_COLLECTIVE:
                nc.sync.dma_start(
                    out=ag_out[(qt, hf)][0:nrow, :], in_=attn_dram[(qt, hf)][:]
                )
                nc.sync.dma_start(
                    out=ag_out[(qt, hf)][nrow : 2 * nrow, :],
                    in_=attn_dram[(qt, hf)][:],
                )
            else:
                nc.gpsimd.collective_compute(
                    "AllGather",
                    mybir.AluOpType.bypass,
                    ins=[attn_dram[(qt, hf)][:]],
                    outs=[ag_out[(qt, hf)][:]],
                    replica_groups=PAIRS,
                )
            # issue the read-back here: it must follow the collective in
            # program order for the RAW dependency to be tracked
            fetch_ag(qt, hf)

        agt_tiles = {}

        def fetch_ag(qt, hf):
            nj = 2 * len(HALF_PAIRS[hf])
            agt = agt_pool.tile([128, nj, TQ], bf16, tag=f"agt{hf}", name="agt")
            nc.sync.dma_start(
                out=agt[:],
                in_=ag_out[(qt, hf)][:].rearrange("(j p) t -> p j t", p=128),
            )
            agt_tiles[(qt, hf)] = agt

        def out_chunk(qt, c4):
            """One 128-col out-projection chunk: 8 accumulating matmuls,
            first 6 contracting the pairs-{0,1,2} AG half, last 2 the rest."""
            cs = slice(c4 * 128, (c4 + 1) * 128)
            ps = proj_ps.tile([128, TQ], f32, tag="proj", name="proj")
            for j in range(NCC):
                hf = 0 if j < 6 else 1
                agt = agt_tiles[(qt, hf)]
                nc.tensor.matmul(
                    ps[:, :],
                    w_sb["o"][:, j, cs],
                    agt[:, j - 6 * hf, :],
                    start=(j == 0),
                    stop=(j == NCC - 1),
                )
            osb = osb_pool.tile([128, TQ], f32, tag="osb", name="osb")
            nc.vector.tensor_copy(out=osb[:], in_=ps[:, :])
            nc.sync.dma_start(
                out=outT[cs, qt * TQ : (qt + 1) * TQ], in_=osb[:]
            )

        # ---- schedule ----
        # Just-in-time interleave: pair p's exp-bound attention window hosts
        # the projection chunks of pair p+1 (same q-tile; pair 0 of the next
        # tile during pair 3) plus one out-projection chunk of qt-1, so the
        # PE stream stays dense and HAM keeps the tensor engine at full rate.
        fetch_x(0)
        for name in ("q", "k", "v"):
            proj_chunk(0, 0, name)
        vtrans_chunk(0, 0)

        pending_apply_b = None
        for qt in range(NT):
            if qt < NT - 1:
                fetch_x(qt + 1)
            g_t = {
                h: gath_pool.tile([128, TQ], f32, tag=f"gather{h}", name="gather")
                for h in (0, 1, 3)
            }
            gr_t = {}
            for p in range(NPAIR):
                win = []
                tp = (qt, p + 1) if p < 3 else (qt + 1, 0)
                if tp[0] < NT:
                    win += [
                        (lambda t=tp[0], pp=tp[1], n=name: proj_chunk(t, pp, n))
                        for name in ("q", "k", "v")
                    ]
                if qt >= 1 and p >= 1:
                    # out-proj of qt-1: not in pair 0's window (its AG-b only
                    # lands a few us into this tile); pair 3 takes two chunks
                    win.append(lambda q=qt - 1, c=p - 1: out_chunk(q, c))
                    if p == 3:
                        win.append(lambda q=qt - 1: out_chunk(q, 3))
                if tp[0] < NT:
                    # vtrans last: needed only by pair p+1's diagonal groups
                    win.append(lambda t=tp[0], pp=tp[1]: vtrans_chunk(t, pp))
                if p == 0 and pending_apply_b is not None:
                    # previous tile's pair-3 normalization + AG: its recip
                    # was issued at tile end and has finished by now
                    win.insert(min(1, len(win)), pending_apply_b)
                    pending_apply_b = None
                if qt == NT - 1 and p == 3:
                    # hold everything back: drained right after the final
                    # reciprocal issues, covering its latency
                    rate = 0.0
                else:
                    rate = len(win) / ((qt + 1) * 4 + 1)
                fi = iter(win)
                acc = 0.0
                for k, _ in enumerate(attn_stream(p, qt, g_t)):
                    if p == 3 and k == 3 * qt + 4:
                        # pairs {0,1,2} normalization mid-window (their recip
                        # ran during earlier yields); AG-a triggers well
                        # before the tile ends
                        norm_apply(qt, gr_t, 0)
                    acc += rate
                    while acc >= 1.0:
                        acc -= 1.0
                        nxt = next(fi, None)
                        if nxt is not None:
                            nxt()
                if p == 2:
                    # pairs {0,1,2} recip runs on DVE during pair 3's window
                    norm_recip(g_t, gr_t, 0)
                elif p == 3:
                    norm_recip(g_t, gr_t, 1)
                for nxt in fi:
                    nxt()
                if p == 3:
                    if qt < NT - 1:
                        pending_apply_b = (
                            lambda q=qt, g=gr_t: norm_apply(q, g, 1)
                        )
                    else:
                        norm_apply(qt, gr_t, 1)
        # tail: contract the big AG half for every column chunk first, so
        # only the 2 pair-3 contraction matmuls per chunk wait on AG-b
        tail_ps = {}
        for c4 in range(4):
            cs = slice(c4 * 128, (c4 + 1) * 128)
            pool, w = (proj_ps, TQ) if c4 < 2 else (sc_ps, GROUP * TQ)
            ps = pool.tile([128, w], f32, tag="proj" if c4 < 2 else "sc", name="tail")
            tail_ps[c4] = ps
            agt = agt_tiles[(NT - 1, 0)]
            for j in range(6):
                nc.tensor.matmul(
                    ps[:, 0:TQ],
                    w_sb["o"][:, j, cs],
                    agt[:, j, :],
                    start=(j == 0),
                    stop=False,
                )
        for c4 in range(4):
            cs = slice(c4 * 128, (c4 + 1) * 128)
            ps = tail_ps[c4]
            agt = agt_tiles[(NT - 1, 1)]
            for j in range(6, NCC):
                nc.tensor.matmul(
                    ps[:, 0:TQ],
                    w_sb["o"][:, j, cs],
                    agt[:, j - 6, :],
                    start=False,
                    stop=(j == NCC - 1),
                )
            osb = osb_pool.tile([128, TQ], f32, tag="osb", name="osb")
            nc.vector.tensor_copy(out=osb[:], in_=ps[:, 0:TQ])
            nc.sync.dma_start(
                out=outT[cs, (NT - 1) * TQ : NT * TQ], in_=osb[:]
            )

    _split_multi_waits(nc)
    return nc


def _prepare_inputs(x, W_qkv, W_out):
    import ml_dtypes

    bf16 = ml_dtypes.bfloat16
    band = (np.arange(128)[None, :] >= np.arange(128)[:, None]).astype(bf16)
    ident = np.eye(64, dtype=bf16)
    Wq = W_qkv[:, 0:D_MODEL]
    Wk = W_qkv[:, D_MODEL : 2 * D_MODEL]
    Wv = W_qkv[:, 2 * D_MODEL :]
    # out-proj contraction chunks 0..5 read the pairs-{0,1,2} AG half
    # (attn dims 0:384 from rank0, 512:896 from rank1), 6..7 the rest.
    row_perm = np.concatenate(
        [np.arange(0, 384), np.arange(512, 896), np.arange(384, 512), np.arange(896, 1024)]
    )
    Wo_p = W_out[row_perm, :]
    in_maps = []
    for c in range(N_CORES):
        b, hh = c // 2, c % 2
        hd = slice(512 * hh, 512 * (hh + 1))
        in_maps.append(
            {
                "xT": np.ascontiguousarray(x[b].T).astype(bf16),
                "wq": np.ascontiguousarray(Wq[:, hd]).astype(bf16),
                "wk": np.ascontiguousarray(Wk[:, hd]).astype(bf16),
                "wv": np.ascontiguousarray(Wv[:, hd]).astype(bf16),
                "wo": np.ascontiguousarray(Wo_p[:, hd]).astype(bf16),
                "band": band,
                "ident": ident,
            }
        )
    return in_maps


def run(x, W_qkv, W_out, trace=False):
    import sys

    if "/opt/trn_rl_repo" not in sys.path:
        sys.path.insert(0, "/opt/trn_rl_repo")
    from concourse.bass_utils import run_bass_kernel_spmd

    key = "program"
    if key not in _PROGRAM_CACHE:
        _PROGRAM_CACHE[key] = _build_program()
    nc = _PROGRAM_CACHE[key]
    in_maps = _prepare_inputs(x, W_qkv, W_out)
    res = run_bass_kernel_spmd(
        nc, in_maps, core_ids=list(range(N_CORES)), trace=trace
    )
    out = np.empty((B, T, D_MODEL), dtype=np.float32)
    for c in range(N_CORES):
        b, hh = c // 2, c % 2
        out[b, :, 512 * hh : 512 * (hh + 1)] = res.results[c]["outT"].T
    return out, res


def kernel(x, W_qkv, W_out):
    out, _ = run(
        np.asarray(x, dtype=np.float32),
        np.asarray(W_qkv, dtype=np.float32),
        np.asarray(W_out, dtype=np.float32),
    )
    return out


# revision 46
# speedup vs baseline: 1.1087x; 1.0015x over previous
"""Causal self-attention on 8 trn2 NeuronCores.

Sharding: (batch, head-half) per core. Core c handles batch b=c//2 and
heads hh*8..hh*8+7 where hh=c%2. QKV projection + attention run fully
local in bf16; the two cores of a batch exchange attention-output halves
with pair AllGathers (two per 512-token q-tile, staged per pair-half so
the exchange hides behind later compute); out-projection is
column-parallel within the pair (512 output cols/core); host assembles.

Schedule: the attention inner loop is scalar-engine (exp) bound, so QKV
projection chunks of the next t-tile and out-projection chunks of the
previous q-tile are interleaved between attention score/AV groups to
keep the PE stream dense (avoids HAM down-throttle).

Layout per core (pairs p=0..3, local heads 2p, 2p+1):
  Q2T/K2T[p] [128, T] bf16  transposed q/k head-dim-major.
  VN[head]   [128, 16, 65] bf16 V k-chunks + ones column so the AV
             matmul also emits the softmax row-sum at psum partition 64.
  scores     S^T chunk [128 k, <=512 q] f32 psum; diagonal chunks are
             computed sliced ([o:512]) instead of masked+memset.
  E^T        exp(S/8) bf16 via ACT; band-mask mult on diagonal block.
  attnall[p] [128, T] bf16 unnormalized; batched reciprocal + ones
             broadcast matmul normalize in place per pair-half.
W_out is row-permuted host-side so out-projection contraction chunks
0..3 come from the pairs-{0,1} AllGather and 4..7 from pairs-{2,3}.
"""

import numpy as np

D_MODEL = 1024
N_HEADS = 16
HEAD_DIM = 64
B = 4
T = 2048
N_CORES = 8
TQ = 512          # q tile
KC = 128          # k chunk
GROUP = 2         # k-chunks per exp group
NT = T // TQ      # q tiles per batch (4)
NKC = T // KC     # k chunks per batch (16)
NCC = D_MODEL // 128  # contraction chunks (8)
NPAIR = 4         # head pairs per core (8 heads)
MODEL_NO_COLLECTIVE = False  # timing-analysis only: swap AG for local DMA

_PROGRAM_CACHE = {}


def _split_multi_waits(nc, max_waits=1):
    """This toolchain's walrus encodes at most one sync-wait per
    instruction; hoist excess waits onto same-engine carrier nops."""
    import concourse.mybir as mybir

    ctr = 0
    for f in nc.m.functions:
        new_blocks = []
        for bb in f.blocks:
            insts = list(bb.instructions)
            if not any(
                inst.sync_info is not None and len(inst.sync_info.on_wait) > max_waits
                for inst in insts
            ):
                new_blocks.append(bb)
                continue
            out = []
            for inst in insts:
                si = inst.sync_info
                if si is not None and len(si.on_wait) > max_waits:
                    waits = list(si.on_wait)
                    excess = waits[max_waits:]
                    while excess:
                        ctr += 1
                        nop = mybir.InstNoOp(
                            name=f"waitcarrier-{ctr}", engine=inst.engine
                        )
                        nop.sync_info = mybir.SyncInfo(
                            on_wait=excess[:max_waits], on_update=[]
                        )
                        out.append(nop)
                        excess = excess[max_waits:]
                    si.on_wait = waits[:max_waits]
                out.append(inst)
            nb = mybir.BasicBlock(
                name=bb.name,
                instructions=[],
                IsPredicated=bb.IsPredicated,
                IsExit=bb.IsExit,
                IsLoopEntry=bb.IsLoopEntry,
            )
            for i in out:
                nb.add_instruction(i)
            new_blocks.append(nb)
        f.blocks = new_blocks


def _build_program():
    import concourse.bass as bass
    import concourse.mybir as mybir
    import concourse.tile as tile
    from contextlib import ExitStack

    f32 = mybir.dt.float32
    bf16 = mybir.dt.bfloat16
    nc = bass.Bass()

    xT = nc.declare_dram_parameter("xT", [D_MODEL, T], bf16, isOutput=False)
    wq = nc.declare_dram_parameter("wq", [D_MODEL, TQ], bf16, isOutput=False)
    wk = nc.declare_dram_parameter("wk", [D_MODEL, TQ], bf16, isOutput=False)
    wv = nc.declare_dram_parameter("wv", [D_MODEL, TQ], bf16, isOutput=False)
    wo = nc.declare_dram_parameter("wo", [D_MODEL, TQ], bf16, isOutput=False)
    band_in = nc.declare_dram_parameter("band", [128, 128], bf16, isOutput=False)
    ident_in = nc.declare_dram_parameter("ident", [64, 64], bf16, isOutput=False)
    outT = nc.declare_dram_parameter("outT", [TQ, T], f32, isOutput=True)

    # asymmetric exchange halves: pairs {0,1,2} then {3} so the last
    # collective of a q-tile is as small as possible
    HALF_PAIRS = ((0, 1, 2), (3,))
    attn_dram = {
        (qt, hf): nc.dram_tensor(
            f"attn_d{qt}_{hf}", [128 * len(HALF_PAIRS[hf]), TQ], bf16
        )
        for qt in range(NT)
        for hf in range(2)
    }
    ag_out = {
        (qt, hf): nc.dram_tensor(
            f"ag_out{qt}_{hf}", [256 * len(HALF_PAIRS[hf]), TQ], bf16
        )
        for qt in range(NT)
        for hf in range(2)
    }
    PAIRS = [[0, 1], [2, 3], [4, 5], [6, 7]]

    with tile.TileContext(nc) as tc, ExitStack() as ctx:
        const_pool = ctx.enter_context(tc.tile_pool(name="const", bufs=1))
        qk_pool = ctx.enter_context(tc.tile_pool(name="qk", bufs=1))
        vn_pool = ctx.enter_context(tc.tile_pool(name="vn", bufs=1))
        stream_pool = ctx.enter_context(tc.tile_pool(name="stream", bufs=2))
        agt_pool = ctx.enter_context(tc.tile_pool(name="agt", bufs=4))
        v2t_pool = ctx.enter_context(tc.tile_pool(name="v2t", bufs=2))
        e_pool = ctx.enter_context(tc.tile_pool(name="etile", bufs=4))
        gath_pool = ctx.enter_context(tc.tile_pool(name="gath", bufs=2))
        attnall_pool = ctx.enter_context(tc.tile_pool(name="attnall", bufs=1))
        osb_pool = ctx.enter_context(tc.tile_pool(name="osb", bufs=2))
        proj_ps = ctx.enter_context(tc.tile_pool(name="projps", bufs=2, space="PSUM"))
        sc_ps = ctx.enter_context(tc.tile_pool(name="scps", bufs=2, space="PSUM"))
        av_ps = ctx.enter_context(tc.tile_pool(name="avps", bufs=2, space="PSUM"))

        # ---- weights (wq first + split: first proj matmuls start sooner) ----
        w_sb = {}
        for name, src in (("q", wq), ("k", wk), ("v", wv), ("o", wo)):
            t_ = const_pool.tile([128, NCC, TQ], bf16, tag=f"w{name}", name=f"w{name}")
            r = src.rearrange("(j p) d -> p j d", p=128)
            if name == "q":
                nc.sync.dma_start(out=t_[:, 0:2, :], in_=r[:, 0:2, :])
                nc.sync.dma_start(out=t_[:, 2:4, :], in_=r[:, 2:4, :])
                nc.sync.dma_start(out=t_[:, 4:NCC, :], in_=r[:, 4:NCC, :])
            else:
                nc.sync.dma_start(out=t_[:], in_=r)
            w_sb[name] = t_
        band = const_pool.tile([128, 128], bf16)
        nc.sync.dma_start(out=band[:], in_=band_in[:])
        ident = const_pool.tile([128, 64], bf16)
        nc.sync.dma_start(out=ident[0:64, :], in_=ident_in[:])
        nc.sync.dma_start(out=ident[64:128, :], in_=ident_in[:])
        ones_t = const_pool.tile([128, 64], bf16)
        nc.vector.memset(ones_t[:], 1.0)

        # persistent tiles (one batch per core)
        q2t = [
            qk_pool.tile([128, T], bf16, tag=f"q2t{p}", name=f"q2t{p}")
            for p in range(NPAIR)
        ]
        k2t = [
            qk_pool.tile([128, T], bf16, tag=f"k2t{p}", name=f"k2t{p}")
            for p in range(NPAIR)
        ]
        vn = [
            vn_pool.tile([128, NKC, 65], bf16, tag=f"vn{h}", name=f"vn{h}")
            for h in range(2 * NPAIR)
        ]
        for h in range(2 * NPAIR):
            nc.vector.memset(vn[h][:, :, 64:65], 1.0)
        attnall = [
            attnall_pool.tile([128, T], bf16, tag=f"attnall{p}", name=f"attnall{p}")
            for p in range(NPAIR)
        ]

        xt_tiles = {}

        def fetch_x(tt):
            xt = stream_pool.tile([128, NCC, TQ], bf16, tag="xt", name="xt")
            r = xT[:, tt * TQ : (tt + 1) * TQ].rearrange("(j p) t -> p j t", p=128)
            if tt == 0:
                nc.gpsimd.dma_start(out=xt[:, 0:2, :], in_=r[:, 0:2, :])
                nc.gpsimd.dma_start(out=xt[:, 2:4, :], in_=r[:, 2:4, :])
                nc.gpsimd.dma_start(out=xt[:, 4:NCC, :], in_=r[:, 4:NCC, :])
            else:
                nc.gpsimd.dma_start(out=xt[:], in_=r)
            xt_tiles[tt] = xt

        v2t_tiles = {}

        def proj_chunk(tt, p, name):
            """One 128-col projection chunk: 8 accumulating matmuls.
            PSUM evacuation runs on the (otherwise idle) Pool engine."""
            xt = xt_tiles[tt]
            cs = slice(p * 128, (p + 1) * 128)
            col0 = tt * TQ
            ps = proj_ps.tile([128, TQ], f32, tag="proj", name="proj")
            for j in range(NCC):
                nc.tensor.matmul(
                    ps[:, :],
                    w_sb[name][:, j, cs],
                    xt[:, j, :],
                    start=(j == 0),
                    stop=(j == NCC - 1),
                )
            if name == "v":
                v2t = v2t_pool.tile([128, TQ], bf16, tag="v2t", name="v2t")
                nc.vector.tensor_copy(out=v2t[:], in_=ps[:, :])
                v2t_tiles[(tt, p)] = v2t
            else:
                dst = q2t[p] if name == "q" else k2t[p]
                nc.vector.tensor_copy(out=dst[:, col0 : col0 + TQ], in_=ps[:, :])

        def vtrans_chunk(tt, p):
            """Transpose V^T [64,128] slices -> VN [128,64] chunks (issued a
            little after proj-v so the Pool evacuation has completed)."""
            v2t = v2t_tiles.pop((tt, p))
            kc0 = tt * (TQ // KC)
            for h in range(2):
                pt = proj_ps.tile([128, TQ], f32, tag="proj", name="proj")
                for sc in range(TQ // KC):
                    nc.tensor.transpose(
                        pt[0:128, 32 * sc : 32 * sc + 32].bitcast(bf16),
                        v2t[64 * h : 64 * h + 64, sc * KC : (sc + 1) * KC],
                        ident[64 * h : 64 * h + 64, :],
                    )
                nc.vector.tensor_copy(
                    out=vn[2 * p + h][:, kc0 : kc0 + 4, 0:64],
                    in_=pt[0:128, 0:128]
                    .bitcast(bf16)
                    .rearrange("p (c d) -> p c d", d=64),
                )

        def attn_stream(p, qt, g_t):
            """Attention for q-tile qt, head pair p; yields at pipeline
            flush points so filler PE work can be interleaved. Leaves
            unnormalized attn in attnall[p], row-sums in g_t[h] slot 32p."""
            nk = (qt + 1) * (TQ // KC)
            qsl = slice(qt * TQ, (qt + 1) * TQ)
            groups = [list(range(g, min(g + GROUP, nk))) for g in range(0, nk, GROUP)]
            avp = {
                h: av_ps.tile([128, TQ], f32, tag="av", name="av") for h in range(2)
            }
            hs = slice(0, 64), slice(64, 128)
            pend = []

            def flush_one():
                h, g, ps = pend.pop(0)
                et = e_pool.tile([128, GROUP * TQ], bf16, tag="etile", name="etile")
                run = []

                def flush_run():
                    if not run:
                        return
                    j0, j1 = run[0], run[-1]
                    nc.scalar.activation(
                        out=et[:, j0 * TQ : (j1 + 1) * TQ],
                        in_=ps[:, j0 * TQ : (j1 + 1) * TQ],
                        func=mybir.ActivationFunctionType.Exp,
                        scale=0.125,
                    )
                    run.clear()

                for j, kc in enumerate(g):
                    o = kc * KC - qt * TQ
                    if o < 0:
                        run.append(j)
                        continue
                    flush_run()
                    nc.scalar.activation(
                        out=et[:, j * TQ + o : (j + 1) * TQ],
                        in_=ps[:, j * TQ + o : (j + 1) * TQ],
                        func=mybir.ActivationFunctionType.Exp,
                        scale=0.125,
                    )
                    nc.vector.tensor_mul(
                        et[:, j * TQ + o : j * TQ + o + 128],
                        et[:, j * TQ + o : j * TQ + o + 128],
                        band[:],
                    )
                flush_run()
                for j, kc in enumerate(g):
                    o = max(0, kc * KC - qt * TQ)
                    nc.tensor.matmul(
                        avp[h][0:65, o:TQ],
                        vn[2 * p + h][:, kc, :],
                        et[:, j * TQ + o : (j + 1) * TQ],
                        start=(kc == 0),
                        stop=(kc == nk - 1),
                    )

            for g in groups:
                for h in range(2):
                    ps = sc_ps.tile([128, GROUP * TQ], f32, tag="sc", name="sc")
                    for j, kc in enumerate(g):
                        o = max(0, kc * KC - qt * TQ)
                        nc.tensor.matmul(
                            ps[:, j * TQ + o : (j + 1) * TQ],
                            k2t[p][hs[h], kc * KC : (kc + 1) * KC],
                            q2t[p][hs[h], qt * TQ + o : (qt + 1) * TQ],
                            start=True,
                            stop=True,
                        )
                    pend.append((h, g, ps))
                    while len(pend) > 2:
                        flush_one()
                        yield
            while pend:
                flush_one()
                yield

            # evacuate unnormalized attn + row-sums; free psum asap.
            # pair 3's two row-sums go to one tile (slots 64/96) so the
            # tile-end reciprocal is a single DVE op.
            for h in range(2):
                if p == 3:
                    nc.vector.tensor_copy(
                        out=g_t[3][64 + 32 * h : 65 + 32 * h, :],
                        in_=avp[h][64:65, :],
                    )
                else:
                    nc.vector.tensor_copy(
                        out=g_t[h][32 * p : 32 * p + 1, :], in_=avp[h][64:65, :]
                    )
                nc.vector.tensor_copy(
                    out=attnall[p][hs[h], qsl], in_=avp[h][0:64, :]
                )
            yield

        def norm_recip(g_t, gr_t, hf):
            """DVE part of normalization: reciprocal + bf16 cast."""
            if hf == 0:
                # slots 0,32,64 = pairs 0-2, per-head tiles
                for h in range(2):
                    nc.vector.reciprocal(g_t[h][0:65, :], g_t[h][0:65, :])
                    gr_t[(h, 0)] = gath_pool.tile(
                        [128, TQ], bf16, tag=f"gatr{h}0", name="gatr"
                    )
                    nc.vector.tensor_copy(
                        out=gr_t[(h, 0)][0:65, :], in_=g_t[h][0:65, :]
                    )
            else:
                # pair 3: both heads in one tile (slots 64/96) -> one recip
                nc.vector.reciprocal(g_t[3][64:97, :], g_t[3][64:97, :])
                for h in range(2):
                    gr_t[(h, 1)] = gath_pool.tile(
                        [128, TQ], bf16, tag=f"gatr{h}1", name="gatr"
                    )
                    nc.vector.tensor_copy(
                        out=gr_t[(h, 1)][96:97, :],
                        in_=g_t[3][64 + 32 * h : 65 + 32 * h, :],
                    )

        def norm_apply(qt, gr_t, hf):
            """Normalize pairs of half hf of q-tile qt and stage their AG."""
            qsl = slice(qt * TQ, (qt + 1) * TQ)
            gr = {h: gr_t[(h, hf)] for h in range(2)}
            for idx, p in enumerate(HALF_PAIRS[hf]):
                slot = 32 * p
                rp = sc_ps.tile([128, GROUP * TQ], f32, tag="sc", name="sc")
                for h in range(2):
                    nc.tensor.matmul(
                        rp[64 * h : 64 * h + 64, 0:TQ],
                        ones_t[slot : slot + 1, :],
                        gr[h][slot : slot + 1, :],
                        start=True,
                        stop=True,
                        tile_position=(slot, 64 * h),
                    )
                rsb = gath_pool.tile([128, TQ], bf16, tag="rsb", name="rsb")
                nc.vector.tensor_copy(out=rsb[:, :], in_=rp[0:128, 0:TQ])
                nc.gpsimd.tensor_mul(
                    attnall[p][:, qsl], attnall[p][:, qsl], rsb[:, :]
                )
                nc.sync.dma_start(
                    out=attn_dram[(qt, hf)][idx * 128 : (idx + 1) * 128, :],
                    in_=attnall[p][:, qsl],
                )
            nrow = 128 * len(HALF_PAIRS[hf])
            if MODEL_NO_COLLECTIVE:
                nc.sync.dma_start(
                    out=ag_out[(qt, hf)][0:nrow, :], in_=attn_dram[(qt, hf)][:]
                )
                nc.sync.dma_start(
                    out=ag_out[(qt, hf)][nrow : 2 * nrow, :],
                    in_=attn_dram[(qt, hf)][:],
                )
            else:
                nc.gpsimd.collective_compute(
                    "AllGather",
                    mybir.AluOpType.bypass,
                    ins=[attn_dram[(qt, hf)][:]],
                    outs=[ag_out[(qt, hf)][:]],
                    replica_groups=PAIRS,
                )
            # issue the read-back here: it must follow the collective in
            # program order for the RAW dependency to be tracked
            fetch_ag(qt, hf)

        agt_tiles = {}

        def fetch_ag(qt, hf):
            nj = 2 * len(HALF_PAIRS[hf])
            agt = agt_pool.tile([128, nj, TQ], bf16, tag=f"agt{hf}", name="agt")
            nc.sync.dma_start(
                out=agt[:],
                in_=ag_out[(qt, hf)][:].rearrange("(j p) t -> p j t", p=128),
            )
            agt_tiles[(qt, hf)] = agt

        def out_chunk(qt, c4):
            """One 128-col out-projection chunk: 8 accumulating matmuls,
            first 6 contracting the pairs-{0,1,2} AG half, last 2 the rest."""
            cs = slice(c4 * 128, (c4 + 1) * 128)
            ps = proj_ps.tile([128, TQ], f32, tag="proj", name="proj")
            for j in range(NCC):
                hf = 0 if j < 6 else 1
                agt = agt_tiles[(qt, hf)]
                nc.tensor.matmul(
                    ps[:, :],
                    w_sb["o"][:, j, cs],
                    agt[:, j - 6 * hf, :],
                    start=(j == 0),
                    stop=(j == NCC - 1),
                )
            osb = osb_pool.tile([128, TQ], f32, tag="osb", name="osb")
            nc.vector.tensor_copy(out=osb[:], in_=ps[:, :])
            nc.sync.dma_start(
                out=outT[cs, qt * TQ : (qt + 1) * TQ], in_=osb[:]
            )

        # ---- schedule ----
        # Just-in-time interleave: pair p's exp-bound attention window hosts
        # the projection chunks of pair p+1 (same q-tile; pair 0 of the next
        # tile during pair 3) plus one out-projection chunk of qt-1, so the
        # PE stream stays dense and HAM keeps the tensor engine at full rate.
        fetch_x(0)
        for name in ("q", "k", "v"):
            proj_chunk(0, 0, name)
        vtrans_chunk(0, 0)

        pending_apply_b = None
        for qt in range(NT):
            if qt < NT - 1:
                fetch_x(qt + 1)
            g_t = {
                h: gath_pool.tile([128, TQ], f32, tag=f"gather{h}", name="gather")
                for h in (0, 1, 3)
            }
            gr_t = {}
            for p in range(NPAIR):
                win = []
                tp = (qt, p + 1) if p < 3 else (qt + 1, 0)
                if tp[0] < NT:
                    win += [
                        (lambda t=tp[0], pp=tp[1], n=name: proj_chunk(t, pp, n))
                        for name in ("q", "k", "v")
                    ]
                if qt >= 1 and p >= 1:
                    # out-proj of qt-1: not in pair 0's window (its AG-b only
                    # lands a few us into this tile); pair 3 takes two chunks
                    win.append(lambda q=qt - 1, c=p - 1: out_chunk(q, c))
                    if p == 3:
                        win.append(lambda q=qt - 1: out_chunk(q, 3))
                if tp[0] < NT:
                    # vtrans last: needed only by pair p+1's diagonal groups
                    win.append(lambda t=tp[0], pp=tp[1]: vtrans_chunk(t, pp))
                if p == 0 and pending_apply_b is not None:
                    # previous tile's pair-3 normalization + AG: its recip
                    # was issued at tile end and has finished by now
                    win.insert(min(1, len(win)), pending_apply_b)
                    pending_apply_b = None
                if qt == NT - 1 and p == 3:
                    # hold everything back: drained right after the final
                    # reciprocal issues, covering its latency
                    rate = 0.0
                else:
                    rate = len(win) / ((qt + 1) * 4 + 1)
                fi = iter(win)
                acc = 0.0
                for k, _ in enumerate(attn_stream(p, qt, g_t)):
                    if p == 3 and k == 2 * qt + 4:
                        # pairs {0,1,2} normalization mid-window (their recip
                        # ran during earlier yields); AG-a triggers well
                        # before the tile ends
                        norm_apply(qt, gr_t, 0)
                    acc += rate
                    while acc >= 1.0:
                        acc -= 1.0
                        nxt = next(fi, None)
                        if nxt is not None:
                            nxt()
                if p == 2:
                    # pairs {0,1,2} recip runs on DVE during pair 3's window
                    norm_recip(g_t, gr_t, 0)
                elif p == 3:
                    norm_recip(g_t, gr_t, 1)
                for nxt in fi:
                    nxt()
                if p == 3:
                    if qt < NT - 1:
                        pending_apply_b = (
                            lambda q=qt, g=gr_t: norm_apply(q, g, 1)
                        )
                    else:
                        norm_apply(qt, gr_t, 1)
        # tail: contract the big AG half for every column chunk first, so
        # only the 2 pair-3 contraction matmuls per chunk wait on AG-b
        tail_ps = {}
        for c4 in range(4):
            cs = slice(c4 * 128, (c4 + 1) * 128)
            pool, w = (proj_ps, TQ) if c4 < 2 else (sc_ps, GROUP * TQ)
            ps = pool.tile([128, w], f32, tag="proj" if c4 < 2 else "sc", name="tail")
            tail_ps[c4] = ps
            agt = agt_tiles[(NT - 1, 0)]
            for j in range(6):
                nc.tensor.matmul(
                    ps[:, 0:TQ],
                    w_sb["o"][:, j, cs],
                    agt[:, j, :],
                    start=(j == 0),
                    stop=False,
                )
        for c4 in range(4):
            cs = slice(c4 * 128, (c4 + 1) * 128)
            ps = tail_ps[c4]
            agt = agt_tiles[(NT - 1, 1)]
            for j in range(6, NCC):
                nc.tensor.matmul(
                    ps[:, 0:TQ],
                    w_sb["o"][:, j, cs],
                    agt[:, j - 6, :],
                    start=False,
                    stop=(j == NCC - 1),
                )
            osb = osb_pool.tile([128, TQ], f32, tag="osb", name="osb")
            nc.vector.tensor_copy(out=osb[:], in_=ps[:, 0:TQ])
            nc.sync.dma_start(
                out=outT[cs, (NT - 1) * TQ : NT * TQ], in_=osb[:]
            )

    _split_multi_waits(nc)
    return nc


def _prepare_inputs(x, W_qkv, W_out):
    import ml_dtypes

    bf16 = ml_dtypes.bfloat16
    band = (np.arange(128)[None, :] >= np.arange(128)[:, None]).astype(bf16)
    ident = np.eye(64, dtype=bf16)
    Wq = W_qkv[:, 0:D_MODEL]
    Wk = W_qkv[:, D_MODEL : 2 * D_MODEL]
    Wv = W_qkv[:, 2 * D_MODEL :]
    # out-proj contraction chunks 0..5 read the pairs-{0,1,2} AG half
    # (attn dims 0:384 from rank0, 512:896 from rank1), 6..7 the rest.
    row_perm = np.concatenate(
        [np.arange(0, 384), np.arange(512, 896), np.arange(384, 512), np.arange(896, 1024)]
    )
    Wo_p = W_out[row_perm, :]
    in_maps = []
    for c in range(N_CORES):
        b, hh = c // 2, c % 2
        hd = slice(512 * hh, 512 * (hh + 1))
        in_maps.append(
            {
                "xT": np.ascontiguousarray(x[b].T).astype(bf16),
                "wq": np.ascontiguousarray(Wq[:, hd]).astype(bf16),
                "wk": np.ascontiguousarray(Wk[:, hd]).astype(bf16),
                "wv": np.ascontiguousarray(Wv[:, hd]).astype(bf16),
                "wo": np.ascontiguousarray(Wo_p[:, hd]).astype(bf16),
                "band": band,
                "ident": ident,
            }
        )
    return in_maps


def run(x, W_qkv, W_out, trace=False):
    import sys

    if "/opt/trn_rl_repo" not in sys.path:
        sys.path.insert(0, "/opt/trn_rl_repo")
    from concourse.bass_utils import run_bass_kernel_spmd

    key = "program"
    if key not in _PROGRAM_CACHE:
        _PROGRAM_CACHE[key] = _build_program()
    nc = _PROGRAM_CACHE[key]
    in_maps = _prepare_inputs(x, W_qkv, W_out)
    res = run_bass_kernel_spmd(
        nc, in_maps, core_ids=list(range(N_CORES)), trace=trace
    )
    out = np.empty((B, T, D_MODEL), dtype=np.float32)
    for c in range(N_CORES):
        b, hh = c // 2, c % 2
        out[b, :, 512 * hh : 512 * (hh + 1)] = res.results[c]["outT"].T
    return out, res


def kernel(x, W_qkv, W_out):
    out, _ = run(
        np.asarray(x, dtype=np.float32),
        np.asarray(W_qkv, dtype=np.float32),
        np.asarray(W_out, dtype=np.float32),
    )
    return out
